# revision 5
# baseline (speedup 1.0000x reference)
"""DeepGraphInfomax loss (2-layer GCN encoder, pos+neg, DGI readout) on 8 trn2 cores.

Strategy (dst-sharded pull-mode GNN aggregation, PSUM-direct):
  - Nodes (dst rows) sharded contiguously across 8 cores (12500 each).
  - pos/neg feature streams fused into 128-wide rows: X2[r] = [x[r] | x[perm[r]]].
  - W1/W2 applied *after* aggregation (A @ (X W) == (A @ X) W).
  - Edges sorted by (dst-tile, src-section, src); dst tiles are 112 wide so
    the mean edge count per (tile, section) cell (448) sits just below the
    512 = 4x128 subtile quantization boundary; each cell is padded to
    128-slot subtiles with a slot count uniform across the 8 cores (SPMD).
  - Supergroups of 7 dst-tiles x 4 section gather-calls (dma_gather, 256B
    bf16 rows, int16 indices sectioned <32768); each dst-tile accumulates
    its subtiles directly in a dedicated full PSUM bank as
    psum[feat, dst] += gt[slot, feat]^T @ P[slot, dst],
    P = (iota == dstl) * norm (one DVE tensor_scalar per subtile).
  - Self-loops bypass the gather entirely: the core's own rows are loaded
    with a direct per-tile DMA (contiguous) and folded in as one extra
    subtile with P = diag-onehot * (1/deg); this also keeps the cross-core
    max padding free of per-core section skew.
  - Post per tile: ACT copies psum -> SBUF bf16, W matmul, ACT bias(+relu),
    DMA-transpose to row-major r2shard (layer 1) or ACT write into the
    feature-major z accumulator (layer 2).
  - Layer-2 sources exchanged with a 2-chunk AllGather of relu(out1) (bf16),
    chunks aligned to gather sections, so the first chunk overlaps the tail
    of layer-1 compute.
  - DGI readout (summary / W_dgi / softplus losses) on device with two tiny
    AllReduces.

Host-side preprocessing only manipulates integer graph structure (degree
counts, sorting, padding, index wrapping) and stages dtype-cast copies of
the inputs; all floating-point math of the reference runs on device.

Empirical notes (measured on HW): single_packet must stay False (True
wedges the device); gather-buffer pool depth 3 is the sweet spot (more
outstanding gathers stall the SWDGE descriptor ring); keep gather calls
at <= ~5400 slots (bigger merged calls are 4x slower).
"""

import sys

for _p in ("/opt/trn_rl_repo", "/root/.axon_site/_ro/trn_rl_repo"):
    if _p not in sys.path:
        sys.path.insert(0, _p)

from contextlib import ExitStack

import ml_dtypes
import numpy as np

import concourse.bass as bass
import concourse.bacc as bacc
import concourse.mybir as mybir
import concourse.tile as tile
from concourse.bass_utils import run_bass_kernel_spmd

BF16 = ml_dtypes.bfloat16
F32 = np.float32

C = 8            # cores
D = 64           # hidden dim
DF = 2 * D       # fused pos|neg width
NSEC = 4
TW = 112         # dst-tile width (mean edges/cell = 448, off the 512 boundary)
SG = 7           # dst-tiles per supergroup (one PSUM bank each)
PAD_DEG = 1e30   # pad-slot degree product -> norm ~ 1e-15 ~ 0


class Geo:
    def __init__(self, npc, nreal):
        self.npc = npc                      # real nodes per core
        self.nreal = nreal                  # total real nodes (= 8*npc)
        self.nt = -(-npc // TW)             # dst tiles per core (112)
        self.ldim = TW * self.nt            # padded dsts per core (12544)
        self.xrows = 8 * self.ldim          # padded source-row space (100352)
        self.sec = self.xrows // NSEC       # 25088 (< 32768 for int16 idx)
        self.nsg = -(-self.nt // SG)        # supergroups (14)
        # allgather chunking aligned to gather sections: chunk 0 rows map to
        # sections 0-2 of the regathered space, chunk 1 to section 3.
        self.ch0 = (NSEC - 1) * self.sec // C   # 9408 rows per core
        self.ch1 = self.ldim - self.ch0         # 3136 rows per core
        assert self.sec < 32768
        assert C * self.ch1 == self.sec


def _src2_map(g, r):
    """Global node id -> virtual row in the 2-chunk allgathered r2 space:
    rows [0, 8*ch0) live in r2cat0 (sections 0-2), rows [8*ch0, xrows) in
    r2cat1 (section 3)."""
    kk = r // g.npc
    d = r % g.npc
    return np.where(d < g.ch0, kk * g.ch0 + d,
                    C * g.ch0 + kk * g.ch1 + (d - g.ch0))


def _preprocess(g, x, W1, b1, W2, b2, W_dgi, edge_index, perm):
    """Build per-core device inputs. Integer index work + dtype staging only."""
    row = np.asarray(edge_index[0], dtype=np.int64)
    col = np.asarray(edge_index[1], dtype=np.int64)
    perm = np.asarray(perm, dtype=np.int64)
    N = g.nreal

    deg = np.bincount(col, minlength=N).astype(np.int64) + 1  # in-deg + 1
    deg_f = deg.astype(np.float64)

    # fused bf16 feature rows, padded to xrows
    X2 = np.zeros((g.xrows, DF), dtype=BF16)
    X2[:N, :D] = x.astype(BF16)
    X2[:N, D:] = x[perm].astype(BF16)

    core_of = col // g.npc
    dst_loc = col - core_of * g.npc

    src_l = [row, _src2_map(g, row)]

    # per-core, per-layer edge arrays; self-loops handled separately via a
    # direct (non-gather) DMA of the core's own contiguous rows per tile.
    per_core = []
    for k in range(C):
        m = core_of == k
        dk = dst_loc[m]
        degp_e = (deg_f[row[m]] * deg_f[col[m]]).astype(F32)
        layers = []
        for li in range(2):
            srcs = src_l[li][m]
            t_arr = dk // TW
            s_arr = srcs // g.sec
            order = np.lexsort((srcs, s_arr, t_arr))
            layers.append((srcs[order], dk[order], degp_e[order],
                           (t_arr * NSEC + s_arr)[order]))
        per_core.append(layers)

    # uniform subtile counts ST[li][t][s] = max over cores
    ST = []
    for li in range(2):
        st = np.zeros((g.nt, NSEC), dtype=np.int64)
        for k in range(C):
            key = per_core[k][li][3]
            cnt = np.bincount(key, minlength=g.nt * NSEC).reshape(g.nt, NSEC)
            st = np.maximum(st, -(-cnt // 128))
        ST.append(st)

    # per-layer layout: subtile order = (sg, s, t-in-sg, j)
    layouts = []
    for li in range(2):
        st = ST[li]
        sub_off = np.zeros((g.nt, NSEC), dtype=np.int64)  # global subtile col
        call_slots = []                                   # per (sg, s) slots
        csub = 0
        for gsg in range(g.nsg):
            t0, t1 = gsg * SG, min((gsg + 1) * SG, g.nt)
            for s in range(NSEC):
                ns = 0
                for t in range(t0, t1):
                    sub_off[t, s] = csub + ns
                    ns += st[t, s]
                csub += ns
                call_slots.append(int(ns) * 128)
        layouts.append((sub_off, call_slots, int(csub)))

    # self-loop metadata: one extra column per dst tile appended after the
    # edge-subtile columns; dl = slot index, degp = deg^2 (PAD for pad dsts)
    dl_self = np.tile(
        np.arange(128, dtype=F32).reshape(128, 1), (1, g.nt)
    )
    dl_self[TW:, :] = -1.0

    # fill per-core slot arrays
    ins = []
    for k in range(C):
        d_in = {}
        dgs = np.full(g.nt * 128, PAD_DEG, dtype=np.float64)
        for t in range(g.nt):
            e = min(128, g.ldim - t * TW)
            dd = np.full(128, PAD_DEG, dtype=np.float64)
            lo = k * g.npc + t * TW
            n = max(0, min(TW, g.npc - t * TW))
            dd[:n] = deg_f[lo : lo + n] ** 2
            dgs[t * 128 : t * 128 + 128] = dd
        dp_self = dgs.reshape(g.nt, 128).T.astype(F32)
        for li in range(2):
            srcs, dsts, dgp, key = per_core[k][li]
            sub_off, call_slots, sttot = layouts[li]
            S = sttot * 128
            idx = np.zeros(S, dtype=np.int16)
            dl = np.zeros(S, dtype=F32)
            dp = np.full(S, PAD_DEG, dtype=F32)
            # ranges per (t, s) in the sorted arrays
            bounds = np.searchsorted(key, np.arange(g.nt * NSEC + 1))
            for t in range(g.nt):
                for s in range(NSEC):
                    lo = bounds[t * NSEC + s]
                    hi = bounds[t * NSEC + s + 1]
                    n = hi - lo
                    if n == 0:
                        continue
                    o = sub_off[t, s] * 128
                    idx[o:o + n] = (srcs[lo:hi] - s * g.sec).astype(np.int16)
                    dl[o:o + n] = (dsts[lo:hi] - t * TW).astype(F32)
                    dp[o:o + n] = dgp[lo:hi]
            # wrapped int16 idx: per 16-slot col, replicated x8 partitions
            idx_w = np.tile(idx.reshape(-1, 16).T, (8, 1)).astype(np.int16)
            L = li + 1
            d_in[f"idx{L}"] = np.ascontiguousarray(idx_w)
            d_in[f"dl{L}"] = np.ascontiguousarray(
                np.concatenate([dl.reshape(-1, 128).T, dl_self], axis=1)
            )
            d_in[f"degp{L}"] = np.ascontiguousarray(
                np.concatenate([dp.reshape(-1, 128).T, dp_self], axis=1)
            )

        # per-core own rows for the self-loop term (row-major, direct DMA)
        d_in["xself"] = np.ascontiguousarray(X2[k * g.npc : k * g.npc + g.ldim])
        # valid-dst mask [128, nt] for the readout (rows >= TW invalid)
        mk = np.zeros((128, g.nt), dtype=F32)
        for t in range(g.nt):
            n = max(0, min(TW, g.npc - t * TW))
            mk[:n, t] = 1.0
        d_in["mask"] = np.ascontiguousarray(mk)
        ins.append(d_in)

    # shared constants
    iota = np.tile(np.arange(128, dtype=F32), (128, 1)).astype(BF16)
    wc1 = np.zeros((DF, DF), dtype=BF16)
    wc1[:D, :D] = W1.astype(BF16)
    wc1[D:, D:] = W1.astype(BF16)
    wc2 = np.zeros((DF, DF), dtype=BF16)
    wc2[:D, :D] = W2.astype(BF16)
    wc2[D:, D:] = W2.astype(BF16)
    bc1 = np.concatenate([b1, b1]).astype(F32).reshape(DF, 1)
    bc2 = np.concatenate([b2, b2]).astype(F32).reshape(DF, 1)
    wstack = np.zeros((D, DF), dtype=F32)
    wstack[:, :D] = W_dgi.T
    wstack[:, D:] = W_dgi.T
    colmask = np.zeros((DF, 2), dtype=F32)
    colmask[:D, 0] = 1.0
    colmask[D:, 1] = 1.0
    nvalid_last = g.npc - (g.nt - 1) * TW
    lastmask = np.tile((np.arange(TW) < nvalid_last).astype(F32), (128, 1))
    shared = {
        "x2": X2,
        "iota": iota,
        "wc1": wc1,
        "wc2": wc2,
        "bc1": bc1,
        "bc2": bc2,
        "wstack": wstack,
        "colmask": colmask,
        "lastmask": lastmask,
        "ones": np.ones((128, 1), dtype=F32),
    }
    for d_in in ins:
        d_in.update(shared)
    meta = (
        tuple(map(tuple, ST[0])), tuple(map(tuple, ST[1])),
    )
    return ins, (ST, layouts, meta)


def _build(g, ST, layouts):
    dt = mybir.dt
    nc = bacc.Bacc(
        "TRN2", target_bir_lowering=False, debug=False, num_devices=C
    )

    def din(name, shape, dty):
        return nc.dram_tensor(name, list(shape), dty, kind="ExternalInput").ap()

    sttot = [layouts[0][2], layouts[1][2]]
    x2 = din("x2", (g.xrows, DF), dt.bfloat16)
    xself_d = din("xself", (g.ldim, DF), dt.bfloat16)
    idx_d = [
        din("idx1", (128, sttot[0] * 8), dt.int16),
        din("idx2", (128, sttot[1] * 8), dt.int16),
    ]
    dl_d = [
        din("dl1", (128, sttot[0] + g.nt), dt.float32),
        din("dl2", (128, sttot[1] + g.nt), dt.float32),
    ]
    degp_d = [
        din("degp1", (128, sttot[0] + g.nt), dt.float32),
        din("degp2", (128, sttot[1] + g.nt), dt.float32),
    ]
    mask_d = din("mask", (128, g.nt), dt.float32)
    iota_d = din("iota", (128, 128), dt.bfloat16)
    wc_d = [din("wc1", (DF, DF), dt.bfloat16), din("wc2", (DF, DF), dt.bfloat16)]
    bc_d = [din("bc1", (DF, 1), dt.float32), din("bc2", (DF, 1), dt.float32)]
    wstack_d = din("wstack", (D, DF), dt.float32)
    colmask_d = din("colmask", (DF, 2), dt.float32)
    lastmask_d = din("lastmask", (128, TW), dt.float32)
    ones_d = din("ones", (128, 1), dt.float32)
    loss_out = nc.dram_tensor(
        "loss", [1, 16], dt.float32, kind="ExternalOutput"
    ).ap()

    inv_n = 1.0 / float(g.nreal)
    rg = [list(range(C))]

    with tile.TileContext(nc) as tc, ExitStack() as ctx:
        dram = ctx.enter_context(tc.tile_pool(name="dram", bufs=1, space="DRAM"))
        r2shard = dram.tile([g.ldim, DF], dt.bfloat16, tag="r2shard")
        r2cat0 = dram.tile(
            [C * g.ch0, DF], dt.bfloat16, tag="r2cat0", addr_space="Shared"
        )
        r2cat1 = dram.tile(
            [C * g.ch1, DF], dt.bfloat16, tag="r2cat1", addr_space="Shared"
        )
        cs_in = dram.tile([128, 1], dt.float32, tag="cs_in")
        cs_out = dram.tile([128, 1], dt.float32, tag="cs_out", addr_space="Shared")
        ls_in = dram.tile([1, 16], dt.float32, tag="ls_in")
        ls_out = dram.tile([1, 16], dt.float32, tag="ls_out", addr_space="Shared")

        const = ctx.enter_context(tc.tile_pool(name="const", bufs=1))

        def cload(ap_dram, shape, dty, tag):
            t = const.tile(list(shape), dty, tag=tag)
            nc.sync.dma_start(t[:], ap_dram)
            return t

        iota_sb = cload(iota_d, (128, 128), dt.bfloat16, "iota")
        wc_sb = [
            cload(wc_d[0], (DF, DF), dt.bfloat16, "wc1"),
            cload(wc_d[1], (DF, DF), dt.bfloat16, "wc2"),
        ]
        bc_sb = [
            cload(bc_d[0], (DF, 1), dt.float32, "bc1"),
            cload(bc_d[1], (DF, 1), dt.float32, "bc2"),
        ]
        wstack_sb = cload(wstack_d, (D, DF), dt.float32, "wstack")
        colmask_sb = cload(colmask_d, (DF, 2), dt.float32, "colmask")
        lastmask_sb = cload(lastmask_d, (128, TW), dt.float32, "lastmask")
        ones_sb = cload(ones_d, (128, 1), dt.float32, "ones")
        mask_sb = cload(mask_d, (128, g.nt), dt.float32, "mask")

        big = ctx.enter_context(tc.tile_pool(name="big", bufs=1))
        z_sb = big.tile([128, g.ldim], dt.float32, tag="z_sb")

        # per-layer metadata (tags shared between layers; sequential use)
        meta = ctx.enter_context(tc.tile_pool(name="meta", bufs=1))

        idxp = ctx.enter_context(tc.tile_pool(name="idxp", bufs=4))
        # max subtiles in one gather call (for gt buffer shape)
        stmax = 0
        for li in range(2):
            for cs in layouts[li][1]:
                stmax = max(stmax, cs // 128)
        gpool = ctx.enter_context(tc.tile_pool(name="gpool", bufs=3))
        xsp = ctx.enter_context(tc.tile_pool(name="xsp", bufs=2 * SG))
        ppool = ctx.enter_context(tc.tile_pool(name="ppool", bufs=6))
        upool = ctx.enter_context(tc.tile_pool(name="upool", bufs=3))
        outp = ctx.enter_context(tc.tile_pool(name="outp", bufs=3))
        psa = ctx.enter_context(tc.tile_pool(name="psa", bufs=1, space="PSUM"))
        pso = ctx.enter_context(tc.tile_pool(name="pso", bufs=1, space="PSUM"))

        def layer(li, to_r2):
            st, (sub_off, call_slots, sttot_l) = ST[li], layouts[li]
            dlt = meta.tile([128, sttot_l + g.nt], dt.float32, tag="dl")
            nc.sync.dma_start(dlt[:], dl_d[li])
            wv = meta.tile([128, sttot_l + g.nt], dt.float32, tag="wv")
            nc.sync.dma_start(wv[:], degp_d[li])
            nc.vector.reciprocal(wv[:], wv[:])
            nc.scalar.sqrt(wv[:], wv[:])
            self_src = xself_d if li == 0 else r2shard[:]

            def src_sec(s):
                if li == 0:
                    return x2[s * g.sec : (s + 1) * g.sec, :]
                if s < NSEC - 1:
                    return r2cat0[s * g.sec : (s + 1) * g.sec, :]
                return r2cat1[:]

            for gsg in range(g.nsg):
                t0, t1 = gsg * SG, min((gsg + 1) * SG, g.nt)
                accs = {}
                xs = {}
                for t in range(t0, t1):
                    accs[t] = psa.tile(
                        [128, 512], dt.float32, tag=f"acc{t - t0}",
                        name=f"acc{t - t0}",
                    )
                    xs[t] = xsp.tile(
                        [128, DF], dt.bfloat16, tag="xs", name="xs"
                    )
                    e = min(128, g.ldim - t * TW)
                    nc.sync.dma_start(
                        xs[t][0:e, :], self_src[t * TW : t * TW + e, :]
                    )
                for s in range(NSEC):
                    slots = call_slots[gsg * NSEC + s]
                    if slots == 0:
                        continue
                    nsub = slots // 128
                    base_sub = sub_off[t0, s]
                    it = idxp.tile([128, stmax * 8], dt.int16, tag="it")
                    nc.sync.dma_start(
                        it[:, : nsub * 8],
                        idx_d[li][:, base_sub * 8 : (base_sub + nsub) * 8],
                    )
                    gt = gpool.tile([128, stmax, DF], dt.bfloat16, tag="gt")
                    nc.gpsimd.dma_gather(
                        gt[:, :nsub, :],
                        src_sec(s),
                        it[:, : nsub * 8],
                        slots,
                        slots,
                        DF,
                        single_packet=False,
                    )
                    for t in range(t0, t1):
                        for j in range(st[t, s]):
                            col = sub_off[t, s] + j
                            off = col - base_sub
                            P = ppool.tile([128, TW], dt.bfloat16, tag="P")
                            nc.vector.tensor_scalar(
                                P[:],
                                iota_sb[:, 0:TW],
                                dlt[:, col : col + 1],
                                wv[:, col : col + 1],
                                mybir.AluOpType.is_equal,
                                mybir.AluOpType.mult,
                            )
                            first = True
                            for s2 in range(NSEC):
                                if st[t, s2] > 0:
                                    first = s2 == s and j == 0
                                    break
                            nc.tensor.matmul(
                                accs[t][:, 0:TW],
                                lhsT=gt[:, off, :],
                                rhs=P[:],
                                start=first,
                                stop=False,
                            )
                # self-loop term + post per tile
                for t in range(t0, t1):
                    col = sttot_l + t
                    e = min(128, g.ldim - t * TW)
                    P = ppool.tile([128, TW], dt.bfloat16, tag="P")
                    nc.vector.tensor_scalar(
                        P[:],
                        iota_sb[:, 0:TW],
                        dlt[:, col : col + 1],
                        wv[:, col : col + 1],
                        mybir.AluOpType.is_equal,
                        mybir.AluOpType.mult,
                    )
                    nc.tensor.matmul(
                        accs[t][:, 0:TW],
                        lhsT=xs[t][0:e, :],
                        rhs=P[0:e, :],
                        start=all(st[t, s] == 0 for s in range(NSEC)),
                        stop=True,
                    )
                    u = upool.tile([128, TW], dt.bfloat16, tag="u")
                    nc.scalar.activation(
                        u[:],
                        accs[t][:, 0:TW],
                        mybir.ActivationFunctionType.Copy,
                    )
                    po = pso.tile([128, 512], dt.float32, tag="po")
                    nc.tensor.matmul(
                        po[:, 0:TW],
                        lhsT=wc_sb[li][:],
                        rhs=u[:],
                        start=True,
                        stop=True,
                    )
                    sl = slice(t * TW, (t + 1) * TW)
                    if to_r2:
                        rb = outp.tile([128, 128], dt.bfloat16, tag="rb")
                        nc.scalar.activation(
                            rb[:, 0:TW],
                            po[:, 0:TW],
                            mybir.ActivationFunctionType.Relu,
                            bias=bc_sb[li][:],
                        )
                        rt = outp.tile([128, 128], dt.bfloat16, tag="rt")
                        nc.sync.dma_start_transpose(rt[:], rb[:])
                        nc.sync.dma_start(r2shard[sl, :], rt[0:TW, :])
                    else:
                        nc.scalar.activation(
                            z_sb[:, sl],
                            po[:, 0:TW],
                            mybir.ActivationFunctionType.Identity,
                            bias=bc_sb[li][:],
                        )
                        if t == g.nt - 1:
                            nc.vector.tensor_tensor(
                                z_sb[:, sl],
                                z_sb[:, sl],
                                lastmask_sb[:],
                                op=mybir.AluOpType.mult,
                            )

        layer(0, to_r2=True)

        nc.gpsimd.collective_compute(
            "AllGather",
            mybir.AluOpType.bypass,
            replica_groups=rg,
            ins=[r2shard[0 : g.ch0, :].opt()],
            outs=[r2cat0[:].opt()],
        )
        nc.gpsimd.collective_compute(
            "AllGather",
            mybir.AluOpType.bypass,
            replica_groups=rg,
            ins=[r2shard[g.ch0 : g.ldim, :].opt()],
            outs=[r2cat1[:].opt()],
        )

        layer(1, to_r2=False)

        # ---- DGI readout ----
        fin = ctx.enter_context(tc.tile_pool(name="fin", bufs=1))
        cs = fin.tile([128, 1], dt.float32, tag="cs")
        nc.vector.reduce_sum(cs[:], z_sb[:], axis=mybir.AxisListType.X)
        nc.sync.dma_start(cs_in[:], cs[:])
        nc.gpsimd.collective_compute(
            "AllReduce",
            mybir.AluOpType.add,
            replica_groups=rg,
            ins=[cs_in[:].opt()],
            outs=[cs_out[:].opt()],
        )
        cst = fin.tile([128, 1], dt.float32, tag="cst")
        nc.sync.dma_start(cst[:], cs_out[:])
        summ = fin.tile([128, 1], dt.float32, tag="summ")
        nc.scalar.activation(
            summ[:], cst[:], mybir.ActivationFunctionType.Sigmoid, scale=inv_n
        )
        wsps = pso.tile([128, 512], dt.float32, tag="po")
        nc.tensor.matmul(
            wsps[0:DF, 0:1], lhsT=wstack_sb[:], rhs=summ[0:D, 0:1],
            start=True, stop=True,
        )
        ws2 = fin.tile([DF, 2], dt.float32, tag="ws2")
        nc.vector.tensor_tensor(
            ws2[:],
            colmask_sb[:],
            wsps[0:DF, 0:1].to_broadcast([DF, 2]),
            op=mybir.AluOpType.mult,
        )
        tp_sb = fin.tile([128, g.nt], dt.float32, tag="tp_sb")
        tn_sb = fin.tile([128, g.nt], dt.float32, tag="tn_sb")
        nc.vector.memset(tp_sb[:], 0.0)
        nc.vector.memset(tn_sb[:], 0.0)
        for dti in range(g.nt):
            sl = slice(dti * TW, (dti + 1) * TW)
            tps = pso.tile([128, 512], dt.float32, tag="po")
            nc.tensor.matmul(
                tps[0:TW, 0:2], lhsT=z_sb[:, sl], rhs=ws2[:], start=True,
                stop=True,
            )
            nc.vector.tensor_copy(tp_sb[0:TW, dti : dti + 1], tps[0:TW, 0:1])
            nc.vector.tensor_copy(tn_sb[0:TW, dti : dti + 1], tps[0:TW, 1:2])

        # softplus(sgn*t) = relu(sgn*t) + ln1p(exp(-|t|)); deg-7 poly for ln1p
        LN1P = [
            5.62195900721818e-07, 0.9999574870750696, -0.4992065685478763,
            0.32697310001391783, -0.2228362583278401, 0.13076503250360005,
            -0.05262485136716543, 0.010119082927575069,
        ]

        def softplus_of(t_in, sgn, tagp):
            neg = fin.tile([128, g.nt], dt.float32, tag=f"{tagp}neg")
            nc.vector.tensor_scalar(
                neg[:], t_in[:], -1.0, None, mybir.AluOpType.mult
            )
            ab = fin.tile([128, g.nt], dt.float32, tag=f"{tagp}ab")
            nc.vector.tensor_tensor(ab[:], t_in[:], neg[:], op=mybir.AluOpType.max)
            uu = fin.tile([128, g.nt], dt.float32, tag=f"{tagp}uu")
            nc.scalar.activation(
                uu[:], ab[:], mybir.ActivationFunctionType.Exp, scale=-1.0
            )
            pp_ = fin.tile([128, g.nt], dt.float32, tag=f"{tagp}pp")
            nc.vector.tensor_scalar(
                pp_[:], uu[:], LN1P[7], LN1P[6],
                mybir.AluOpType.mult, mybir.AluOpType.add,
            )
            pm = fin.tile([128, g.nt], dt.float32, tag=f"{tagp}pm")
            for ci in range(5, -1, -1):
                nc.vector.tensor_tensor(
                    pm[:], pp_[:], uu[:], op=mybir.AluOpType.mult
                )
                nc.vector.tensor_scalar(
                    pp_[:], pm[:], LN1P[ci], None, mybir.AluOpType.add
                )
            rl = fin.tile([128, g.nt], dt.float32, tag=f"{tagp}rl")
            nc.vector.tensor_scalar(
                rl[:], (t_in if sgn > 0 else neg)[:], 0.0, None,
                mybir.AluOpType.max,
            )
            res = fin.tile([128, g.nt], dt.float32, tag=f"{tagp}res")
            nc.vector.tensor_tensor(res[:], rl[:], pp_[:], op=mybir.AluOpType.add)
            return res

        spp = softplus_of(tp_sb, -1, "sp")   # softplus(-t_pos)
        spn = softplus_of(tn_sb, +1, "sn")   # softplus(t_neg)
        ssum = fin.tile([128, g.nt], dt.float32, tag="ssum")
        nc.vector.tensor_tensor(ssum[:], spp[:], spn[:], op=mybir.AluOpType.add)
        nc.vector.tensor_tensor(
            ssum[:], ssum[:], mask_sb[:], op=mybir.AluOpType.mult
        )
        srow = fin.tile([128, 1], dt.float32, tag="srow")
        nc.vector.reduce_sum(srow[:], ssum[:], axis=mybir.AxisListType.X)
        tot = pso.tile([128, 512], dt.float32, tag="po")
        nc.tensor.matmul(
            tot[0:1, 0:1], lhsT=srow[:], rhs=ones_sb[:], start=True, stop=True
        )
        lsb = fin.tile([1, 16], dt.float32, tag="lsb")
        nc.vector.memset(lsb[:], 0.0)
        nc.vector.tensor_copy(lsb[0:1, 0:1], tot[0:1, 0:1])
        nc.sync.dma_start(ls_in[:], lsb[:])
        nc.gpsimd.collective_compute(
            "AllReduce",
            mybir.AluOpType.add,
            replica_groups=rg,
            ins=[ls_in[:].opt()],
            outs=[ls_out[:].opt()],
        )
        lsf = fin.tile([1, 16], dt.float32, tag="lsf")
        nc.sync.dma_start(lsf[:], ls_out[:])
        lout = fin.tile([1, 16], dt.float32, tag="lout")
        nc.scalar.activation(
            lout[:], lsf[:], mybir.ActivationFunctionType.Copy, scale=inv_n
        )
        nc.sync.dma_start(loss_out, lout[:])

    nc.compile()
    return nc


_prog_cache = {}


def _get_prog(g, ST, layouts, meta):
    key = (g.npc, g.nreal, meta)
    if key not in _prog_cache:
        _prog_cache[key] = _build(g, ST, layouts)
    return _prog_cache[key]


def run(inputs, npc=12500, nreal=100000, trace=False):
    g = Geo(npc, nreal)
    in_maps, (ST, layouts, meta) = _preprocess(g, **inputs)
    nc = _get_prog(g, ST, layouts, meta)
    res = run_bass_kernel_spmd(
        nc, in_maps, core_ids=list(range(C)), trace=trace
    )
    loss = res.results[0]["loss"][0, 0]
    return np.float32(loss), res


def kernel(**inputs):
    out, _ = run(inputs)
    return out


def _make_sharded_exec(nc, in_maps, reps=1):
    """Jitted shard_map executor mirroring bass2jax's multi-core path, with
    device-resident inputs."""
    import jax
    from jax.experimental.shard_map import shard_map
    from jax.sharding import Mesh, NamedSharding, PartitionSpec

    from concourse import bass2jax, mybir as _mb

    bass2jax.install_neuronx_cc_hook()
    partition_name = (
        nc.partition_id_tensor.name if nc.partition_id_tensor else None
    )
    in_names, out_names, out_avals, zero_shapes = [], [], [], []
    for alloc in nc.m.functions[0].allocations:
        if not isinstance(alloc, _mb.MemoryLocationSet):
            continue
        name = alloc.memorylocations[0].name
        if alloc.kind == "ExternalInput":
            if name != partition_name:
                in_names.append(name)
        elif alloc.kind == "ExternalOutput":
            shape = tuple(alloc.tensor_shape)
            dty = _mb.dt.np(alloc.dtype)
            out_names.append(name)
            out_avals.append(jax.core.ShapedArray(shape, dty))
            zero_shapes.append((shape, dty))
    n_params = len(in_names)
    n_outs = len(out_avals)
    all_names = list(in_names) + list(out_names)
    if partition_name is not None:
        all_names.append(partition_name)
    donate = tuple(range(n_params, n_params + n_outs * reps))

    assert reps == 1

    def _body(*args):
        operands = list(args)
        if partition_name is not None:
            operands.append(bass2jax.partition_id_tensor())
        outs = bass2jax._bass_exec_p.bind(
            *operands,
            out_avals=tuple(out_avals),
            in_names=tuple(all_names),
            out_names=tuple(out_names),
            lowering_input_output_aliases=(),
            sim_require_finite=True,
            sim_require_nnan=True,
            nc=nc,
        )
        return tuple(outs)

    devices = jax.devices()[:C]
    mesh = Mesh(np.array(devices), ("core",))
    spec = PartitionSpec("core")
    sharded = jax.jit(
        shard_map(
            _body,
            mesh=mesh,
            in_specs=(spec,) * (n_params + n_outs * reps),
            out_specs=(spec,) * n_outs,
            check_rep=False,
        ),
        donate_argnums=donate,
        keep_unused=True,
    )
    shard = NamedSharding(mesh, spec)
    concat_in = [
        jax.device_put(
            np.concatenate([np.asarray(m[nm]) for m in in_maps], axis=0), shard
        )
        for nm in in_names
    ]

    def launch():
        zeros = [
            jax.device_put(np.zeros((C * s[0], *s[1:]), d), shard)
            for (s, d) in zero_shapes
        ]
        return sharded(*concat_in, *zeros)

    def fetch(outs):
        jax.block_until_ready(outs)
        return {
            nm: np.asarray(outs[i]).reshape(C, *out_avals[i].shape)[0]
            for i, nm in enumerate(out_names)
        }

    def run_once():
        return fetch(launch())

    run_once.launch = launch
    run_once.fetch = fetch
    return run_once


def bench(inputs, npc=12500, nreal=100000, iters=6):
    import time

    g = Geo(npc, nreal)
    t0 = time.time()
    in_maps, (ST, layouts, meta) = _preprocess(g, **inputs)
    t1 = time.time()
    nc = _get_prog(g, ST, layouts, meta)
    t2 = time.time()
    run_1 = _make_sharded_exec(nc, in_maps)
    out = run_1()  # warmup: compiles + loads NEFF
    t3 = time.time()
    t1s = []
    for _ in range(iters):
        ta = time.time()
        out = run_1()
        t1s.append(time.time() - ta)
    # Pipelined-launch marginal: executions overlap the ~150ms axon
    # round-trip, so the added time per extra launch approximates on-device
    # execution. The kernel is now near the measurement noise floor
    # (a trivial NEFF measures ~0.4ms by the same method), so take the
    # median of several repetitions.
    import statistics
    import jax as _jax
    pers = []
    for _rep in range(9):
        K = 16
        ta = time.time()
        pend = [run_1.launch() for _ in range(K)]
        _jax.block_until_ready(pend)
        tK = time.time() - ta
        pers.append((tK - min(t1s)) / (K - 1))
    per = max(statistics.median(pers), 1e-6)
    print(f"  marginal reps ms: {[round(p*1e3,3) for p in pers]}")
    print(
        f"preprocess {t1-t0:.1f}s  build {t2-t1:.1f}s  warmup {t3-t2:.1f}s\n"
        f"  1-shot ms: {[round(t*1e3,2) for t in t1s]}\n"
        f"  {K} pipelined: total {tK*1e3:.1f} ms -> marginal {per*1e3:.3f} ms"
    )
    return np.float32(out["loss"][0, 0]), per


# revision 6
# speedup vs baseline: 1.2584x; 1.2584x over previous
"""DeepGraphInfomax loss (2-layer GCN encoder, pos+neg, DGI readout) on 8 trn2 cores.

Strategy (dst-sharded pull-mode GNN aggregation, PSUM-direct):
  - Nodes (dst rows) sharded contiguously across 8 cores (12500 each).
  - pos/neg feature streams fused into 128-wide rows: X2[r] = [x[r] | x[perm[r]]].
  - W1/W2 applied *after* aggregation (A @ (X W) == (A @ X) W).
  - Edges sorted by (dst-tile, src-section, src); dst tiles are 112 wide so
    the mean edge count per (tile, section) cell (448) sits just below the
    512 = 4x128 subtile quantization boundary; each cell is padded to
    128-slot subtiles with a slot count uniform across the 8 cores (SPMD).
  - Supergroups of 7 dst-tiles x 4 section gather-calls (dma_gather, 256B
    bf16 rows, int16 indices sectioned <32768); each dst-tile accumulates
    its subtiles directly in a dedicated full PSUM bank as
    psum[feat, dst] += gt[slot, feat]^T @ P[slot, dst],
    P = (iota == dstl) * norm (one DVE tensor_scalar per subtile).
  - Self-loops bypass the gather entirely: the core's own rows are loaded
    with a direct per-tile DMA (contiguous) and folded in as one extra
    subtile with P = diag-onehot * (1/deg); this also keeps the cross-core
    max padding free of per-core section skew.
  - Post per tile: ACT copies psum -> SBUF bf16, W matmul, ACT bias(+relu),
    DMA-transpose to row-major r2shard (layer 1) or ACT write into the
    feature-major z accumulator (layer 2).
  - Layer-2 sources exchanged with a 2-chunk AllGather of relu(out1) (bf16),
    chunks aligned to gather sections, so the first chunk overlaps the tail
    of layer-1 compute.
  - DGI readout (summary / W_dgi / softplus losses) on device with two tiny
    AllReduces.

Host-side preprocessing only manipulates integer graph structure (degree
counts, sorting, padding, index wrapping) and stages dtype-cast copies of
the inputs; all floating-point math of the reference runs on device.

Empirical notes (measured on HW): single_packet must stay False (True
wedges the device); gather-buffer pool depth 3 is the sweet spot (more
outstanding gathers stall the SWDGE descriptor ring); keep gather calls
at <= ~5400 slots (bigger merged calls are 4x slower).
"""

import sys

for _p in ("/opt/trn_rl_repo", "/root/.axon_site/_ro/trn_rl_repo"):
    if _p not in sys.path:
        sys.path.insert(0, _p)

from contextlib import ExitStack

import ml_dtypes
import numpy as np

import concourse.bass as bass
import concourse.bacc as bacc
import concourse.mybir as mybir
import concourse.tile as tile
from concourse.bass_utils import run_bass_kernel_spmd

BF16 = ml_dtypes.bfloat16
F32 = np.float32

C = 8            # cores
D = 64           # hidden dim
DF = 2 * D       # fused pos|neg width
NSEC = 4
TW = 112         # dst-tile width (mean edges/cell = 448, off the 512 boundary)
SG = 7           # dst-tiles per supergroup (one PSUM bank each)
PAD_DEG = 1e30   # pad-slot degree product -> norm ~ 1e-15 ~ 0


class Geo:
    def __init__(self, npc, nreal):
        self.npc = npc                      # real nodes per core
        self.nreal = nreal                  # total real nodes (= 8*npc)
        self.nt = -(-npc // TW)             # dst tiles per core (112)
        self.ldim = TW * self.nt            # padded dsts per core (12544)
        self.xrows = 8 * self.ldim          # padded source-row space (100352)
        self.sec = self.xrows // NSEC       # 25088 (< 32768 for int16 idx)
        self.nsg = -(-self.nt // SG)        # supergroups (14)
        # allgather chunking aligned to gather sections: chunk 0 rows map to
        # sections 0-2 of the regathered space, chunk 1 to section 3.
        self.ch0 = (NSEC - 1) * self.sec // C   # 9408 rows per core
        self.ch1 = self.ldim - self.ch0         # 3136 rows per core
        assert self.sec < 32768
        assert C * self.ch1 == self.sec


def _src2_map(g, r):
    """Global node id -> virtual row in the 2-chunk allgathered r2 space:
    rows [0, 8*ch0) live in r2cat0 (sections 0-2), rows [8*ch0, xrows) in
    r2cat1 (section 3)."""
    kk = r // g.npc
    d = r % g.npc
    return np.where(d < g.ch0, kk * g.ch0 + d,
                    C * g.ch0 + kk * g.ch1 + (d - g.ch0))


def _preprocess(g, x, W1, b1, W2, b2, W_dgi, edge_index, perm):
    """Build per-core device inputs. Integer index work + dtype staging only."""
    row = np.asarray(edge_index[0], dtype=np.int64)
    col = np.asarray(edge_index[1], dtype=np.int64)
    perm = np.asarray(perm, dtype=np.int64)
    N = g.nreal

    deg = np.bincount(col, minlength=N).astype(np.int64) + 1  # in-deg + 1
    deg_f = deg.astype(np.float64)

    # fused bf16 feature rows, padded to xrows
    X2 = np.zeros((g.xrows, DF), dtype=BF16)
    X2[:N, :D] = x.astype(BF16)
    X2[:N, D:] = x[perm].astype(BF16)

    core_of = col // g.npc
    dst_loc = col - core_of * g.npc

    src_l = [row, _src2_map(g, row)]

    # per-core, per-layer edge arrays; self-loops handled separately via a
    # direct (non-gather) DMA of the core's own contiguous rows per tile.
    per_core = []
    for k in range(C):
        m = core_of == k
        dk = dst_loc[m]
        degp_e = (deg_f[row[m]] * deg_f[col[m]]).astype(F32)
        layers = []
        for li in range(2):
            srcs = src_l[li][m]
            t_arr = dk // TW
            s_arr = srcs // g.sec
            order = np.lexsort((srcs, s_arr, t_arr))
            layers.append((srcs[order], dk[order], degp_e[order],
                           (t_arr * NSEC + s_arr)[order]))
        per_core.append(layers)

    # uniform subtile counts ST[li][t][s] = max over cores
    ST = []
    for li in range(2):
        st = np.zeros((g.nt, NSEC), dtype=np.int64)
        for k in range(C):
            key = per_core[k][li][3]
            cnt = np.bincount(key, minlength=g.nt * NSEC).reshape(g.nt, NSEC)
            st = np.maximum(st, -(-cnt // 128))
        ST.append(st)

    # per-layer layout: subtile order = (sg, s, t-in-sg, j)
    layouts = []
    for li in range(2):
        st = ST[li]
        sub_off = np.zeros((g.nt, NSEC), dtype=np.int64)  # global subtile col
        call_slots = []                                   # per (sg, s) slots
        csub = 0
        for gsg in range(g.nsg):
            t0, t1 = gsg * SG, min((gsg + 1) * SG, g.nt)
            for s in range(NSEC):
                ns = 0
                for t in range(t0, t1):
                    sub_off[t, s] = csub + ns
                    ns += st[t, s]
                csub += ns
                call_slots.append(int(ns) * 128)
        layouts.append((sub_off, call_slots, int(csub)))

    # self-loop metadata: one extra column per dst tile appended after the
    # edge-subtile columns; dl = slot index, degp = deg^2 (PAD for pad dsts)
    dl_self = np.tile(
        np.arange(128, dtype=F32).reshape(128, 1), (1, g.nt)
    )
    dl_self[TW:, :] = -1.0

    # fill per-core slot arrays
    ins = []
    for k in range(C):
        d_in = {}
        dgs = np.full(g.nt * 128, PAD_DEG, dtype=np.float64)
        for t in range(g.nt):
            e = min(128, g.ldim - t * TW)
            dd = np.full(128, PAD_DEG, dtype=np.float64)
            lo = k * g.npc + t * TW
            n = max(0, min(TW, g.npc - t * TW))
            dd[:n] = deg_f[lo : lo + n] ** 2
            dgs[t * 128 : t * 128 + 128] = dd
        dp_self = dgs.reshape(g.nt, 128).T.astype(F32)
        for li in range(2):
            srcs, dsts, dgp, key = per_core[k][li]
            sub_off, call_slots, sttot = layouts[li]
            S = sttot * 128
            idx = np.zeros(S, dtype=np.int16)
            dl = np.zeros(S, dtype=F32)
            dp = np.full(S, PAD_DEG, dtype=F32)
            # ranges per (t, s) in the sorted arrays
            bounds = np.searchsorted(key, np.arange(g.nt * NSEC + 1))
            for t in range(g.nt):
                for s in range(NSEC):
                    lo = bounds[t * NSEC + s]
                    hi = bounds[t * NSEC + s + 1]
                    n = hi - lo
                    if n == 0:
                        continue
                    o = sub_off[t, s] * 128
                    idx[o:o + n] = (srcs[lo:hi] - s * g.sec).astype(np.int16)
                    dl[o:o + n] = (dsts[lo:hi] - t * TW).astype(F32)
                    dp[o:o + n] = dgp[lo:hi]
            # wrapped int16 idx: per 16-slot col, replicated x8 partitions
            idx_w = np.tile(idx.reshape(-1, 16).T, (8, 1)).astype(np.int16)
            L = li + 1
            d_in[f"idx{L}"] = np.ascontiguousarray(idx_w)
            d_in[f"dl{L}"] = np.ascontiguousarray(
                np.concatenate([dl.reshape(-1, 128).T, dl_self], axis=1)
            )
            d_in[f"degp{L}"] = np.ascontiguousarray(
                np.concatenate([dp.reshape(-1, 128).T, dp_self], axis=1)
            )

        # per-core own rows for the self-loop term (row-major, direct DMA)
        d_in["xself"] = np.ascontiguousarray(X2[k * g.npc : k * g.npc + g.ldim])
        # valid-dst mask [128, nt] for the readout (rows >= TW invalid)
        mk = np.zeros((128, g.nt), dtype=F32)
        for t in range(g.nt):
            n = max(0, min(TW, g.npc - t * TW))
            mk[:n, t] = 1.0
        d_in["mask"] = np.ascontiguousarray(mk)
        ins.append(d_in)

    # shared constants
    iota = np.tile(np.arange(128, dtype=F32), (128, 1)).astype(BF16)
    wc1 = np.zeros((DF, DF), dtype=BF16)
    wc1[:D, :D] = W1.astype(BF16)
    wc1[D:, D:] = W1.astype(BF16)
    wc2 = np.zeros((DF, DF), dtype=BF16)
    wc2[:D, :D] = W2.astype(BF16)
    wc2[D:, D:] = W2.astype(BF16)
    bc1 = np.concatenate([b1, b1]).astype(F32).reshape(DF, 1)
    bc2 = np.concatenate([b2, b2]).astype(F32).reshape(DF, 1)
    wstack = np.zeros((D, DF), dtype=F32)
    wstack[:, :D] = W_dgi.T
    wstack[:, D:] = W_dgi.T
    colmask = np.zeros((DF, 2), dtype=F32)
    colmask[:D, 0] = 1.0
    colmask[D:, 1] = 1.0
    nvalid_last = g.npc - (g.nt - 1) * TW
    lastmask = np.tile((np.arange(TW) < nvalid_last).astype(F32), (128, 1))
    shared = {
        "x2": X2,
        "iota": iota,
        "wc1": wc1,
        "wc2": wc2,
        "bc1": bc1,
        "bc2": bc2,
        "wstack": wstack,
        "colmask": colmask,
        "lastmask": lastmask,
        "ones": np.ones((128, 1), dtype=F32),
    }
    for d_in in ins:
        d_in.update(shared)
    meta = (
        tuple(map(tuple, ST[0])), tuple(map(tuple, ST[1])),
    )
    return ins, (ST, layouts, meta)


def _build(g, ST, layouts):
    dt = mybir.dt
    nc = bacc.Bacc(
        "TRN2", target_bir_lowering=False, debug=False, num_devices=C
    )

    def din(name, shape, dty):
        return nc.dram_tensor(name, list(shape), dty, kind="ExternalInput").ap()

    sttot = [layouts[0][2], layouts[1][2]]
    x2 = din("x2", (g.xrows, DF), dt.bfloat16)
    xself_d = din("xself", (g.ldim, DF), dt.bfloat16)
    idx_d = [
        din("idx1", (128, sttot[0] * 8), dt.int16),
        din("idx2", (128, sttot[1] * 8), dt.int16),
    ]
    dl_d = [
        din("dl1", (128, sttot[0] + g.nt), dt.float32),
        din("dl2", (128, sttot[1] + g.nt), dt.float32),
    ]
    degp_d = [
        din("degp1", (128, sttot[0] + g.nt), dt.float32),
        din("degp2", (128, sttot[1] + g.nt), dt.float32),
    ]
    mask_d = din("mask", (128, g.nt), dt.float32)
    iota_d = din("iota", (128, 128), dt.bfloat16)
    wc_d = [din("wc1", (DF, DF), dt.bfloat16), din("wc2", (DF, DF), dt.bfloat16)]
    bc_d = [din("bc1", (DF, 1), dt.float32), din("bc2", (DF, 1), dt.float32)]
    wstack_d = din("wstack", (D, DF), dt.float32)
    colmask_d = din("colmask", (DF, 2), dt.float32)
    lastmask_d = din("lastmask", (128, TW), dt.float32)
    ones_d = din("ones", (128, 1), dt.float32)
    loss_out = nc.dram_tensor(
        "loss", [1, 16], dt.float32, kind="ExternalOutput"
    ).ap()

    inv_n = 1.0 / float(g.nreal)
    rg = [list(range(C))]

    with tile.TileContext(nc) as tc, ExitStack() as ctx:
        dram = ctx.enter_context(tc.tile_pool(name="dram", bufs=1, space="DRAM"))
        r2shard = dram.tile([g.ldim, DF], dt.bfloat16, tag="r2shard")
        r2cat0 = dram.tile(
            [C * g.ch0, DF], dt.bfloat16, tag="r2cat0", addr_space="Shared"
        )
        r2cat1 = dram.tile(
            [C * g.ch1, DF], dt.bfloat16, tag="r2cat1", addr_space="Shared"
        )
        cs_in = dram.tile([128, 1], dt.float32, tag="cs_in")
        cs_out = dram.tile([128, 1], dt.float32, tag="cs_out", addr_space="Shared")
        ls_in = dram.tile([1, 16], dt.float32, tag="ls_in")
        ls_out = dram.tile([1, 16], dt.float32, tag="ls_out", addr_space="Shared")

        const = ctx.enter_context(tc.tile_pool(name="const", bufs=1))

        def cload(ap_dram, shape, dty, tag):
            t = const.tile(list(shape), dty, tag=tag)
            nc.sync.dma_start(t[:], ap_dram)
            return t

        iota_sb = cload(iota_d, (128, 128), dt.bfloat16, "iota")
        wc_sb = [
            cload(wc_d[0], (DF, DF), dt.bfloat16, "wc1"),
            cload(wc_d[1], (DF, DF), dt.bfloat16, "wc2"),
        ]
        bc_sb = [
            cload(bc_d[0], (DF, 1), dt.float32, "bc1"),
            cload(bc_d[1], (DF, 1), dt.float32, "bc2"),
        ]
        wstack_sb = cload(wstack_d, (D, DF), dt.float32, "wstack")
        colmask_sb = cload(colmask_d, (DF, 2), dt.float32, "colmask")
        lastmask_sb = cload(lastmask_d, (128, TW), dt.float32, "lastmask")
        ones_sb = cload(ones_d, (128, 1), dt.float32, "ones")
        mask_sb = cload(mask_d, (128, g.nt), dt.float32, "mask")

        big = ctx.enter_context(tc.tile_pool(name="big", bufs=1))
        z_sb = big.tile([128, g.ldim], dt.float32, tag="z_sb")

        # per-layer metadata (tags shared between layers; sequential use)
        meta = ctx.enter_context(tc.tile_pool(name="meta", bufs=1))

        idxp = ctx.enter_context(tc.tile_pool(name="idxp", bufs=4))
        # max subtiles in one gather call (for gt buffer shape)
        stmax = 0
        for li in range(2):
            for cs in layouts[li][1]:
                stmax = max(stmax, cs // 128)
        gpool = ctx.enter_context(tc.tile_pool(name="gpool", bufs=3))
        xsp = ctx.enter_context(tc.tile_pool(name="xsp", bufs=2 * SG))
        ppool = ctx.enter_context(tc.tile_pool(name="ppool", bufs=6))
        upool = ctx.enter_context(tc.tile_pool(name="upool", bufs=3))
        outp = ctx.enter_context(tc.tile_pool(name="outp", bufs=3))
        psa = ctx.enter_context(tc.tile_pool(name="psa", bufs=1, space="PSUM"))
        pso = ctx.enter_context(tc.tile_pool(name="pso", bufs=1, space="PSUM"))

        def layer(li, to_r2):
            st, (sub_off, call_slots, sttot_l) = ST[li], layouts[li]
            mtot = sttot_l + g.nt
            msplit = min(int(sub_off[min(SG, g.nt - 1), 0]), mtot)
            dlt = meta.tile([128, mtot], dt.float32, tag="dl")
            wv = meta.tile([128, mtot], dt.float32, tag="wv")
            for a, b in ((0, msplit), (msplit, mtot)):
                if a >= b:
                    continue
                nc.sync.dma_start(dlt[:, a:b], dl_d[li][:, a:b])
                nc.sync.dma_start(wv[:, a:b], degp_d[li][:, a:b])
                nc.vector.reciprocal(wv[:, a:b], wv[:, a:b])
                nc.scalar.sqrt(wv[:, a:b], wv[:, a:b])
            self_src = xself_d if li == 0 else r2shard[:]

            def src_sec(s):
                if li == 0:
                    return x2[s * g.sec : (s + 1) * g.sec, :]
                if s < NSEC - 1:
                    return r2cat0[s * g.sec : (s + 1) * g.sec, :]
                return r2cat1[:]

            for gsg in range(g.nsg):
                t0, t1 = gsg * SG, min((gsg + 1) * SG, g.nt)
                accs = {}
                xs = {}
                for t in range(t0, t1):
                    accs[t] = psa.tile(
                        [128, 512], dt.float32, tag=f"acc{t - t0}",
                        name=f"acc{t - t0}",
                    )
                    xs[t] = xsp.tile(
                        [128, DF], dt.bfloat16, tag="xs", name="xs"
                    )
                    e = min(128, g.ldim - t * TW)
                    nc.sync.dma_start(
                        xs[t][0:e, :], self_src[t * TW : t * TW + e, :]
                    )
                for s in range(NSEC):
                    slots = call_slots[gsg * NSEC + s]
                    if slots == 0:
                        continue
                    nsub = slots // 128
                    base_sub = sub_off[t0, s]
                    it = idxp.tile([128, stmax * 8], dt.int16, tag="it")
                    nc.sync.dma_start(
                        it[:, : nsub * 8],
                        idx_d[li][:, base_sub * 8 : (base_sub + nsub) * 8],
                    )
                    gt = gpool.tile([128, stmax, DF], dt.bfloat16, tag="gt")
                    nc.gpsimd.dma_gather(
                        gt[:, :nsub, :],
                        src_sec(s),
                        it[:, : nsub * 8],
                        slots,
                        slots,
                        DF,
                        single_packet=False,
                    )
                    for t in range(t0, t1):
                        for j in range(st[t, s]):
                            col = sub_off[t, s] + j
                            off = col - base_sub
                            P = ppool.tile([128, TW], dt.bfloat16, tag="P")
                            nc.vector.tensor_scalar(
                                P[:],
                                iota_sb[:, 0:TW],
                                dlt[:, col : col + 1],
                                wv[:, col : col + 1],
                                mybir.AluOpType.is_equal,
                                mybir.AluOpType.mult,
                            )
                            first = True
                            for s2 in range(NSEC):
                                if st[t, s2] > 0:
                                    first = s2 == s and j == 0
                                    break
                            nc.tensor.matmul(
                                accs[t][:, 0:TW],
                                lhsT=gt[:, off, :],
                                rhs=P[:],
                                start=first,
                                stop=False,
                            )
                # self-loop term + post per tile
                for t in range(t0, t1):
                    col = sttot_l + t
                    e = min(128, g.ldim - t * TW)
                    P = ppool.tile([128, TW], dt.bfloat16, tag="P")
                    nc.vector.tensor_scalar(
                        P[:],
                        iota_sb[:, 0:TW],
                        dlt[:, col : col + 1],
                        wv[:, col : col + 1],
                        mybir.AluOpType.is_equal,
                        mybir.AluOpType.mult,
                    )
                    nc.tensor.matmul(
                        accs[t][:, 0:TW],
                        lhsT=xs[t][0:e, :],
                        rhs=P[0:e, :],
                        start=all(st[t, s] == 0 for s in range(NSEC)),
                        stop=True,
                    )
                    u = upool.tile([128, TW], dt.bfloat16, tag="u")
                    nc.scalar.activation(
                        u[:],
                        accs[t][:, 0:TW],
                        mybir.ActivationFunctionType.Copy,
                    )
                    po = pso.tile([128, 512], dt.float32, tag="po")
                    nc.tensor.matmul(
                        po[:, 0:TW],
                        lhsT=wc_sb[li][:],
                        rhs=u[:],
                        start=True,
                        stop=True,
                    )
                    sl = slice(t * TW, (t + 1) * TW)
                    if to_r2:
                        rb = outp.tile([128, 128], dt.bfloat16, tag="rb")
                        nc.scalar.activation(
                            rb[:, 0:TW],
                            po[:, 0:TW],
                            mybir.ActivationFunctionType.Relu,
                            bias=bc_sb[li][:],
                        )
                        rt = outp.tile([128, 128], dt.bfloat16, tag="rt")
                        nc.sync.dma_start_transpose(rt[:], rb[:])
                        nc.sync.dma_start(r2shard[sl, :], rt[0:TW, :])
                    else:
                        nc.scalar.activation(
                            z_sb[:, sl],
                            po[:, 0:TW],
                            mybir.ActivationFunctionType.Identity,
                            bias=bc_sb[li][:],
                        )
                        if t == g.nt - 1:
                            nc.vector.tensor_tensor(
                                z_sb[:, sl],
                                z_sb[:, sl],
                                lastmask_sb[:],
                                op=mybir.AluOpType.mult,
                            )

        layer(0, to_r2=True)

        nc.gpsimd.collective_compute(
            "AllGather",
            mybir.AluOpType.bypass,
            replica_groups=rg,
            ins=[r2shard[0 : g.ch0, :].opt()],
            outs=[r2cat0[:].opt()],
        )
        nc.gpsimd.collective_compute(
            "AllGather",
            mybir.AluOpType.bypass,
            replica_groups=rg,
            ins=[r2shard[g.ch0 : g.ldim, :].opt()],
            outs=[r2cat1[:].opt()],
        )

        layer(1, to_r2=False)

        # ---- DGI readout ----
        fin = ctx.enter_context(tc.tile_pool(name="fin", bufs=1))
        cs = fin.tile([128, 1], dt.float32, tag="cs")
        nc.vector.reduce_sum(cs[:], z_sb[:], axis=mybir.AxisListType.X)
        nc.sync.dma_start(cs_in[:], cs[:])
        nc.gpsimd.collective_compute(
            "AllReduce",
            mybir.AluOpType.add,
            replica_groups=rg,
            ins=[cs_in[:].opt()],
            outs=[cs_out[:].opt()],
        )
        cst = fin.tile([128, 1], dt.float32, tag="cst")
        nc.sync.dma_start(cst[:], cs_out[:])
        summ = fin.tile([128, 1], dt.float32, tag="summ")
        nc.scalar.activation(
            summ[:], cst[:], mybir.ActivationFunctionType.Sigmoid, scale=inv_n
        )
        wsps = pso.tile([128, 512], dt.float32, tag="po")
        nc.tensor.matmul(
            wsps[0:DF, 0:1], lhsT=wstack_sb[:], rhs=summ[0:D, 0:1],
            start=True, stop=True,
        )
        ws2 = fin.tile([DF, 2], dt.float32, tag="ws2")
        nc.vector.tensor_tensor(
            ws2[:],
            colmask_sb[:],
            wsps[0:DF, 0:1].to_broadcast([DF, 2]),
            op=mybir.AluOpType.mult,
        )
        tp_sb = fin.tile([128, g.nt], dt.float32, tag="tp_sb")
        tn_sb = fin.tile([128, g.nt], dt.float32, tag="tn_sb")
        nc.vector.memset(tp_sb[:], 0.0)
        nc.vector.memset(tn_sb[:], 0.0)
        PER_BANK = 32
        for b0 in range(0, g.nt, PER_BANK):
            nb = min(PER_BANK, g.nt - b0)
            tps = pso.tile([128, 512], dt.float32, tag="po")
            for i in range(nb):
                dti = b0 + i
                sl = slice(dti * TW, (dti + 1) * TW)
                nc.tensor.matmul(
                    tps[0:TW, 2 * i : 2 * i + 2],
                    lhsT=z_sb[:, sl],
                    rhs=ws2[:],
                    start=True,
                    stop=True,
                )
            tpv = tps[0:TW, 0 : 2 * nb].rearrange("p (a f) -> p a f", f=2)
            nc.vector.tensor_copy(
                tp_sb[0:TW, b0 : b0 + nb], tpv[:, :, 0:1].opt()
            )
            nc.vector.tensor_copy(
                tn_sb[0:TW, b0 : b0 + nb], tpv[:, :, 1:2].opt()
            )

        # softplus(sgn*t) = relu(sgn*t) + ln1p(exp(-|t|)); deg-7 poly for ln1p
        LN1P = [
            5.62195900721818e-07, 0.9999574870750696, -0.4992065685478763,
            0.32697310001391783, -0.2228362583278401, 0.13076503250360005,
            -0.05262485136716543, 0.010119082927575069,
        ]

        def softplus_of(t_in, sgn, tagp):
            neg = fin.tile([128, g.nt], dt.float32, tag=f"{tagp}neg")
            nc.vector.tensor_scalar(
                neg[:], t_in[:], -1.0, None, mybir.AluOpType.mult
            )
            ab = fin.tile([128, g.nt], dt.float32, tag=f"{tagp}ab")
            nc.vector.tensor_tensor(ab[:], t_in[:], neg[:], op=mybir.AluOpType.max)
            uu = fin.tile([128, g.nt], dt.float32, tag=f"{tagp}uu")
            nc.scalar.activation(
                uu[:], ab[:], mybir.ActivationFunctionType.Exp, scale=-1.0
            )
            pp_ = fin.tile([128, g.nt], dt.float32, tag=f"{tagp}pp")
            nc.vector.tensor_scalar(
                pp_[:], uu[:], LN1P[7], LN1P[6],
                mybir.AluOpType.mult, mybir.AluOpType.add,
            )
            pm = fin.tile([128, g.nt], dt.float32, tag=f"{tagp}pm")
            for ci in range(5, -1, -1):
                nc.vector.tensor_tensor(
                    pm[:], pp_[:], uu[:], op=mybir.AluOpType.mult
                )
                nc.vector.tensor_scalar(
                    pp_[:], pm[:], LN1P[ci], None, mybir.AluOpType.add
                )
            rl = fin.tile([128, g.nt], dt.float32, tag=f"{tagp}rl")
            nc.vector.tensor_scalar(
                rl[:], (t_in if sgn > 0 else neg)[:], 0.0, None,
                mybir.AluOpType.max,
            )
            res = fin.tile([128, g.nt], dt.float32, tag=f"{tagp}res")
            nc.vector.tensor_tensor(res[:], rl[:], pp_[:], op=mybir.AluOpType.add)
            return res

        spp = softplus_of(tp_sb, -1, "sp")   # softplus(-t_pos)
        spn = softplus_of(tn_sb, +1, "sn")   # softplus(t_neg)
        ssum = fin.tile([128, g.nt], dt.float32, tag="ssum")
        nc.vector.tensor_tensor(ssum[:], spp[:], spn[:], op=mybir.AluOpType.add)
        nc.vector.tensor_tensor(
            ssum[:], ssum[:], mask_sb[:], op=mybir.AluOpType.mult
        )
        srow = fin.tile([128, 1], dt.float32, tag="srow")
        nc.vector.reduce_sum(srow[:], ssum[:], axis=mybir.AxisListType.X)
        tot = pso.tile([128, 512], dt.float32, tag="po")
        nc.tensor.matmul(
            tot[0:1, 0:1], lhsT=srow[:], rhs=ones_sb[:], start=True, stop=True
        )
        lsb = fin.tile([1, 16], dt.float32, tag="lsb")
        nc.vector.memset(lsb[:], 0.0)
        nc.vector.tensor_copy(lsb[0:1, 0:1], tot[0:1, 0:1])
        nc.sync.dma_start(ls_in[:], lsb[:])
        nc.gpsimd.collective_compute(
            "AllReduce",
            mybir.AluOpType.add,
            replica_groups=rg,
            ins=[ls_in[:].opt()],
            outs=[ls_out[:].opt()],
        )
        lsf = fin.tile([1, 16], dt.float32, tag="lsf")
        nc.sync.dma_start(lsf[:], ls_out[:])
        lout = fin.tile([1, 16], dt.float32, tag="lout")
        nc.scalar.activation(
            lout[:], lsf[:], mybir.ActivationFunctionType.Copy, scale=inv_n
        )
        nc.sync.dma_start(loss_out, lout[:])

    nc.compile()
    return nc


_prog_cache = {}


def _get_prog(g, ST, layouts, meta):
    key = (g.npc, g.nreal, meta)
    if key not in _prog_cache:
        _prog_cache[key] = _build(g, ST, layouts)
    return _prog_cache[key]


def run(inputs, npc=12500, nreal=100000, trace=False):
    g = Geo(npc, nreal)
    in_maps, (ST, layouts, meta) = _preprocess(g, **inputs)
    nc = _get_prog(g, ST, layouts, meta)
    res = run_bass_kernel_spmd(
        nc, in_maps, core_ids=list(range(C)), trace=trace
    )
    loss = res.results[0]["loss"][0, 0]
    return np.float32(loss), res


def kernel(**inputs):
    out, _ = run(inputs)
    return out


def _make_sharded_exec(nc, in_maps, reps=1):
    """Jitted shard_map executor mirroring bass2jax's multi-core path, with
    device-resident inputs."""
    import jax
    from jax.experimental.shard_map import shard_map
    from jax.sharding import Mesh, NamedSharding, PartitionSpec

    from concourse import bass2jax, mybir as _mb

    bass2jax.install_neuronx_cc_hook()
    partition_name = (
        nc.partition_id_tensor.name if nc.partition_id_tensor else None
    )
    in_names, out_names, out_avals, zero_shapes = [], [], [], []
    for alloc in nc.m.functions[0].allocations:
        if not isinstance(alloc, _mb.MemoryLocationSet):
            continue
        name = alloc.memorylocations[0].name
        if alloc.kind == "ExternalInput":
            if name != partition_name:
                in_names.append(name)
        elif alloc.kind == "ExternalOutput":
            shape = tuple(alloc.tensor_shape)
            dty = _mb.dt.np(alloc.dtype)
            out_names.append(name)
            out_avals.append(jax.core.ShapedArray(shape, dty))
            zero_shapes.append((shape, dty))
    n_params = len(in_names)
    n_outs = len(out_avals)
    all_names = list(in_names) + list(out_names)
    if partition_name is not None:
        all_names.append(partition_name)
    donate = tuple(range(n_params, n_params + n_outs * reps))

    assert reps == 1

    def _body(*args):
        operands = list(args)
        if partition_name is not None:
            operands.append(bass2jax.partition_id_tensor())
        outs = bass2jax._bass_exec_p.bind(
            *operands,
            out_avals=tuple(out_avals),
            in_names=tuple(all_names),
            out_names=tuple(out_names),
            lowering_input_output_aliases=(),
            sim_require_finite=True,
            sim_require_nnan=True,
            nc=nc,
        )
        return tuple(outs)

    devices = jax.devices()[:C]
    mesh = Mesh(np.array(devices), ("core",))
    spec = PartitionSpec("core")
    sharded = jax.jit(
        shard_map(
            _body,
            mesh=mesh,
            in_specs=(spec,) * (n_params + n_outs * reps),
            out_specs=(spec,) * n_outs,
            check_rep=False,
        ),
        donate_argnums=donate,
        keep_unused=True,
    )
    shard = NamedSharding(mesh, spec)
    concat_in = [
        jax.device_put(
            np.concatenate([np.asarray(m[nm]) for m in in_maps], axis=0), shard
        )
        for nm in in_names
    ]

    def launch():
        zeros = [
            jax.device_put(np.zeros((C * s[0], *s[1:]), d), shard)
            for (s, d) in zero_shapes
        ]
        return sharded(*concat_in, *zeros)

    def fetch(outs):
        jax.block_until_ready(outs)
        return {
            nm: np.asarray(outs[i]).reshape(C, *out_avals[i].shape)[0]
            for i, nm in enumerate(out_names)
        }

    def run_once():
        return fetch(launch())

    run_once.launch = launch
    run_once.fetch = fetch
    return run_once


def bench(inputs, npc=12500, nreal=100000, iters=6):
    import time

    g = Geo(npc, nreal)
    t0 = time.time()
    in_maps, (ST, layouts, meta) = _preprocess(g, **inputs)
    t1 = time.time()
    nc = _get_prog(g, ST, layouts, meta)
    t2 = time.time()
    run_1 = _make_sharded_exec(nc, in_maps)
    out = run_1()  # warmup: compiles + loads NEFF
    t3 = time.time()
    t1s = []
    for _ in range(iters):
        ta = time.time()
        out = run_1()
        t1s.append(time.time() - ta)
    # Pipelined-launch marginal: executions overlap the ~150ms axon
    # round-trip, so the added time per extra launch approximates on-device
    # execution. The kernel is now near the measurement noise floor
    # (a trivial NEFF measures ~0.4ms by the same method), so take the
    # median of several repetitions.
    import statistics
    import jax as _jax
    pers = []
    for _rep in range(9):
        K = 16
        ta = time.time()
        pend = [run_1.launch() for _ in range(K)]
        _jax.block_until_ready(pend)
        tK = time.time() - ta
        pers.append((tK - min(t1s)) / (K - 1))
    per = max(statistics.median(pers), 1e-6)
    print(f"  marginal reps ms: {[round(p*1e3,3) for p in pers]}")
    print(
        f"preprocess {t1-t0:.1f}s  build {t2-t1:.1f}s  warmup {t3-t2:.1f}s\n"
        f"  1-shot ms: {[round(t*1e3,2) for t in t1s]}\n"
        f"  {K} pipelined: total {tK*1e3:.1f} ms -> marginal {per*1e3:.3f} ms"
    )
    return np.float32(out["loss"][0, 0]), per


# revision 7
# speedup vs baseline: 294.1930x; 233.7770x over previous
"""DeepGraphInfomax loss (2-layer GCN encoder, pos+neg, DGI readout) on 8 trn2 cores.

Strategy (dst-sharded pull-mode GNN aggregation, PSUM-direct):
  - Nodes (dst rows) sharded contiguously across 8 cores (12500 each).
  - pos/neg feature streams fused into 128-wide rows: X2[r] = [x[r] | x[perm[r]]].
  - W1/W2 applied *after* aggregation (A @ (X W) == (A @ X) W).
  - Edges sorted by (dst-tile, src-section, src); dst tiles are 112 wide so
    the mean edge count per (tile, section) cell (448) sits just below the
    512 = 4x128 subtile quantization boundary; each cell is padded to
    128-slot subtiles with a slot count uniform across the 8 cores (SPMD).
  - Supergroups of 7 dst-tiles x 4 section gather-calls (dma_gather, 256B
    bf16 rows, int16 indices sectioned <32768); each dst-tile accumulates
    its subtiles directly in a dedicated full PSUM bank as
    psum[feat, dst] += gt[slot, feat]^T @ P[slot, dst],
    P = (iota == dstl) * norm (one DVE tensor_scalar per subtile).
  - Self-loops bypass the gather entirely: the core's own rows are loaded
    with a direct per-tile DMA (contiguous) and folded in as one extra
    subtile with P = diag-onehot * (1/deg); this also keeps the cross-core
    max padding free of per-core section skew.
  - Post per tile: ACT copies psum -> SBUF bf16, W matmul, ACT bias(+relu),
    DMA-transpose to row-major r2shard (layer 1) or ACT write into the
    feature-major z accumulator (layer 2).
  - Layer-2 sources exchanged with a 2-chunk AllGather of relu(out1) (bf16),
    chunks aligned to gather sections, so the first chunk overlaps the tail
    of layer-1 compute.
  - DGI readout (summary / W_dgi / softplus losses) on device with two tiny
    AllReduces.

Host-side preprocessing only manipulates integer graph structure (degree
counts, sorting, padding, index wrapping) and stages dtype-cast copies of
the inputs; all floating-point math of the reference runs on device.

Empirical notes (measured on HW): single_packet must stay False (True
wedges the device); gather-buffer pool depth 3 is the sweet spot (more
outstanding gathers stall the SWDGE descriptor ring); keep gather calls
at <= ~5400 slots (bigger merged calls are 4x slower).
"""

import sys

for _p in ("/opt/trn_rl_repo", "/root/.axon_site/_ro/trn_rl_repo"):
    if _p not in sys.path:
        sys.path.insert(0, _p)

from contextlib import ExitStack

import ml_dtypes
import numpy as np

import concourse.bass as bass
import concourse.bacc as bacc
import concourse.mybir as mybir
import concourse.tile as tile
from concourse.bass_utils import run_bass_kernel_spmd

BF16 = ml_dtypes.bfloat16
F32 = np.float32

C = 8            # cores
D = 64           # hidden dim
DF = 2 * D       # fused pos|neg width
NSEC = 4
TW = 112         # dst-tile width (mean edges/cell = 448, off the 512 boundary)
SG = 7           # dst-tiles per supergroup (one PSUM bank each)
PAD_DEG = 1e30   # pad-slot degree product -> norm ~ 1e-15 ~ 0


class Geo:
    def __init__(self, npc, nreal):
        self.npc = npc                      # real nodes per core
        self.nreal = nreal                  # total real nodes (= 8*npc)
        self.nt = -(-npc // TW)             # dst tiles per core (112)
        self.ldim = TW * self.nt            # padded dsts per core (12544)
        self.xrows = 8 * self.ldim          # padded source-row space (100352)
        self.sec = self.xrows // NSEC       # 25088 (< 32768 for int16 idx)
        self.nsg = -(-self.nt // SG)        # supergroups (14)
        # allgather chunking aligned to gather sections: chunk 0 rows map to
        # sections 0-2 of the regathered space, chunk 1 to section 3.
        self.ch0 = (NSEC - 1) * self.sec // C   # 9408 rows per core
        self.ch1 = self.ldim - self.ch0         # 3136 rows per core
        assert self.sec < 32768
        assert C * self.ch1 == self.sec


def _src2_map(g, r):
    """Global node id -> virtual row in the 2-chunk allgathered r2 space:
    rows [0, 8*ch0) live in r2cat0 (sections 0-2), rows [8*ch0, xrows) in
    r2cat1 (section 3)."""
    kk = r // g.npc
    d = r % g.npc
    return np.where(d < g.ch0, kk * g.ch0 + d,
                    C * g.ch0 + kk * g.ch1 + (d - g.ch0))


def _preprocess(g, x, W1, b1, W2, b2, W_dgi, edge_index, perm):
    """Build per-core device inputs. Integer index work + dtype staging only."""
    row = np.asarray(edge_index[0], dtype=np.int64)
    col = np.asarray(edge_index[1], dtype=np.int64)
    perm = np.asarray(perm, dtype=np.int64)
    N = g.nreal

    deg = np.bincount(col, minlength=N).astype(np.int64) + 1  # in-deg + 1
    deg_f = deg.astype(np.float64)

    # fused bf16 feature rows, padded to xrows
    X2 = np.zeros((g.xrows, DF), dtype=BF16)
    X2[:N, :D] = x.astype(BF16)
    X2[:N, D:] = x[perm].astype(BF16)

    core_of = col // g.npc
    dst_loc = col - core_of * g.npc

    src_l = [row, _src2_map(g, row)]

    # per-core, per-layer edge arrays; self-loops handled separately via a
    # direct (non-gather) DMA of the core's own contiguous rows per tile.
    per_core = []
    for k in range(C):
        m = core_of == k
        dk = dst_loc[m]
        degp_e = (deg_f[row[m]] * deg_f[col[m]]).astype(F32)
        layers = []
        for li in range(2):
            srcs = src_l[li][m]
            t_arr = dk // TW
            s_arr = srcs // g.sec
            order = np.lexsort((srcs, s_arr, t_arr))
            layers.append((srcs[order], dk[order], degp_e[order],
                           (t_arr * NSEC + s_arr)[order]))
        per_core.append(layers)

    # uniform subtile counts ST[li][t][s] = max over cores
    ST = []
    for li in range(2):
        st = np.zeros((g.nt, NSEC), dtype=np.int64)
        for k in range(C):
            key = per_core[k][li][3]
            cnt = np.bincount(key, minlength=g.nt * NSEC).reshape(g.nt, NSEC)
            st = np.maximum(st, -(-cnt // 128))
        ST.append(st)

    # per-layer layout: subtile order = (sg, s, t-in-sg, j)
    layouts = []
    for li in range(2):
        st = ST[li]
        sub_off = np.zeros((g.nt, NSEC), dtype=np.int64)  # global subtile col
        call_slots = []                                   # per (sg, s) slots
        csub = 0
        for gsg in range(g.nsg):
            t0, t1 = gsg * SG, min((gsg + 1) * SG, g.nt)
            for s in range(NSEC):
                ns = 0
                for t in range(t0, t1):
                    sub_off[t, s] = csub + ns
                    ns += st[t, s]
                csub += ns
                call_slots.append(int(ns) * 128)
        layouts.append((sub_off, call_slots, int(csub)))

    # self-loop metadata: one extra column per dst tile appended after the
    # edge-subtile columns; dl = slot index, degp = deg^2 (PAD for pad dsts)
    dl_self = np.tile(
        np.arange(128, dtype=F32).reshape(128, 1), (1, g.nt)
    )
    dl_self[TW:, :] = -1.0

    # fill per-core slot arrays
    ins = []
    for k in range(C):
        d_in = {}
        dgs = np.full(g.nt * 128, PAD_DEG, dtype=np.float64)
        for t in range(g.nt):
            e = min(128, g.ldim - t * TW)
            dd = np.full(128, PAD_DEG, dtype=np.float64)
            lo = k * g.npc + t * TW
            n = max(0, min(TW, g.npc - t * TW))
            dd[:n] = deg_f[lo : lo + n] ** 2
            dgs[t * 128 : t * 128 + 128] = dd
        dp_self = dgs.reshape(g.nt, 128).T.astype(F32)
        for li in range(2):
            srcs, dsts, dgp, key = per_core[k][li]
            sub_off, call_slots, sttot = layouts[li]
            S = sttot * 128
            idx = np.zeros(S, dtype=np.int16)
            dl = np.zeros(S, dtype=F32)
            dp = np.full(S, PAD_DEG, dtype=F32)
            # ranges per (t, s) in the sorted arrays
            bounds = np.searchsorted(key, np.arange(g.nt * NSEC + 1))
            for t in range(g.nt):
                for s in range(NSEC):
                    lo = bounds[t * NSEC + s]
                    hi = bounds[t * NSEC + s + 1]
                    n = hi - lo
                    if n == 0:
                        continue
                    o = sub_off[t, s] * 128
                    idx[o:o + n] = (srcs[lo:hi] - s * g.sec).astype(np.int16)
                    dl[o:o + n] = (dsts[lo:hi] - t * TW).astype(F32)
                    dp[o:o + n] = dgp[lo:hi]
            # wrapped int16 idx: per 16-slot col, replicated x8 partitions
            idx_w = np.tile(idx.reshape(-1, 16).T, (8, 1)).astype(np.int16)
            L = li + 1
            d_in[f"idx{L}"] = np.ascontiguousarray(idx_w)
            d_in[f"dl{L}"] = np.ascontiguousarray(
                np.concatenate([dl.reshape(-1, 128).T, dl_self], axis=1)
            )
            d_in[f"degp{L}"] = np.ascontiguousarray(
                np.concatenate([dp.reshape(-1, 128).T, dp_self], axis=1)
            )

        # per-core own rows for the self-loop term (row-major, direct DMA)
        d_in["xself"] = np.ascontiguousarray(X2[k * g.npc : k * g.npc + g.ldim])
        # valid-dst mask [128, nt] for the readout (rows >= TW invalid)
        mk = np.zeros((128, g.nt), dtype=F32)
        for t in range(g.nt):
            n = max(0, min(TW, g.npc - t * TW))
            mk[:n, t] = 1.0
        d_in["mask"] = np.ascontiguousarray(mk)
        ins.append(d_in)

    # shared constants
    iota = np.tile(np.arange(128, dtype=F32), (128, 1)).astype(BF16)
    wc1 = np.zeros((DF, DF), dtype=BF16)
    wc1[:D, :D] = W1.astype(BF16)
    wc1[D:, D:] = W1.astype(BF16)
    wc2 = np.zeros((DF, DF), dtype=BF16)
    wc2[:D, :D] = W2.astype(BF16)
    wc2[D:, D:] = W2.astype(BF16)
    bc1 = np.concatenate([b1, b1]).astype(F32).reshape(DF, 1)
    bc2 = np.concatenate([b2, b2]).astype(F32).reshape(DF, 1)
    wstack = np.zeros((D, DF), dtype=F32)
    wstack[:, :D] = W_dgi.T
    wstack[:, D:] = W_dgi.T
    colmask = np.zeros((DF, 2), dtype=F32)
    colmask[:D, 0] = 1.0
    colmask[D:, 1] = 1.0
    nvalid_last = g.npc - (g.nt - 1) * TW
    lastmask = np.tile((np.arange(TW) < nvalid_last).astype(F32), (128, 1))
    shared = {
        "x2": X2,
        "iota": iota,
        "wc1": wc1,
        "wc2": wc2,
        "bc1": bc1,
        "bc2": bc2,
        "wstack": wstack,
        "colmask": colmask,
        "lastmask": lastmask,
        "ones": np.ones((128, 1), dtype=F32),
    }
    for d_in in ins:
        d_in.update(shared)
    meta = (
        tuple(map(tuple, ST[0])), tuple(map(tuple, ST[1])),
    )
    return ins, (ST, layouts, meta)


def _build(g, ST, layouts):
    dt = mybir.dt
    nc = bacc.Bacc(
        "TRN2", target_bir_lowering=False, debug=False, num_devices=C
    )

    def din(name, shape, dty):
        return nc.dram_tensor(name, list(shape), dty, kind="ExternalInput").ap()

    sttot = [layouts[0][2], layouts[1][2]]
    x2 = din("x2", (g.xrows, DF), dt.bfloat16)
    xself_d = din("xself", (g.ldim, DF), dt.bfloat16)
    idx_d = [
        din("idx1", (128, sttot[0] * 8), dt.int16),
        din("idx2", (128, sttot[1] * 8), dt.int16),
    ]
    dl_d = [
        din("dl1", (128, sttot[0] + g.nt), dt.float32),
        din("dl2", (128, sttot[1] + g.nt), dt.float32),
    ]
    degp_d = [
        din("degp1", (128, sttot[0] + g.nt), dt.float32),
        din("degp2", (128, sttot[1] + g.nt), dt.float32),
    ]
    mask_d = din("mask", (128, g.nt), dt.float32)
    iota_d = din("iota", (128, 128), dt.bfloat16)
    wc_d = [din("wc1", (DF, DF), dt.bfloat16), din("wc2", (DF, DF), dt.bfloat16)]
    bc_d = [din("bc1", (DF, 1), dt.float32), din("bc2", (DF, 1), dt.float32)]
    wstack_d = din("wstack", (D, DF), dt.float32)
    colmask_d = din("colmask", (DF, 2), dt.float32)
    lastmask_d = din("lastmask", (128, TW), dt.float32)
    ones_d = din("ones", (128, 1), dt.float32)
    loss_out = nc.dram_tensor(
        "loss", [1, 16], dt.float32, kind="ExternalOutput"
    ).ap()

    inv_n = 1.0 / float(g.nreal)
    rg = [list(range(C))]

    with tile.TileContext(nc) as tc, ExitStack() as ctx:
        dram = ctx.enter_context(tc.tile_pool(name="dram", bufs=1, space="DRAM"))
        r2shard = dram.tile([g.ldim, DF], dt.bfloat16, tag="r2shard")
        r2cat0 = dram.tile(
            [C * g.ch0, DF], dt.bfloat16, tag="r2cat0", addr_space="Shared"
        )
        r2cat1 = dram.tile(
            [C * g.ch1, DF], dt.bfloat16, tag="r2cat1", addr_space="Shared"
        )
        cs_in = dram.tile([128, 1], dt.float32, tag="cs_in")
        cs_out = dram.tile([128, 1], dt.float32, tag="cs_out", addr_space="Shared")
        ls_in = dram.tile([1, 16], dt.float32, tag="ls_in")
        ls_out = dram.tile([1, 16], dt.float32, tag="ls_out", addr_space="Shared")

        const = ctx.enter_context(tc.tile_pool(name="const", bufs=1))

        def cload(ap_dram, shape, dty, tag):
            t = const.tile(list(shape), dty, tag=tag)
            nc.sync.dma_start(t[:], ap_dram)
            return t

        iota_sb = cload(iota_d, (128, 128), dt.bfloat16, "iota")
        wc_sb = [
            cload(wc_d[0], (DF, DF), dt.bfloat16, "wc1"),
            cload(wc_d[1], (DF, DF), dt.bfloat16, "wc2"),
        ]
        bc_sb = [
            cload(bc_d[0], (DF, 1), dt.float32, "bc1"),
            cload(bc_d[1], (DF, 1), dt.float32, "bc2"),
        ]
        wstack_sb = cload(wstack_d, (D, DF), dt.float32, "wstack")
        colmask_sb = cload(colmask_d, (DF, 2), dt.float32, "colmask")
        lastmask_sb = cload(lastmask_d, (128, TW), dt.float32, "lastmask")
        ones_sb = cload(ones_d, (128, 1), dt.float32, "ones")
        mask_sb = cload(mask_d, (128, g.nt), dt.float32, "mask")

        big = ctx.enter_context(tc.tile_pool(name="big", bufs=1))
        z_sb = big.tile([128, g.ldim], dt.float32, tag="z_sb")

        # per-layer metadata (tags shared between layers; sequential use)
        meta = ctx.enter_context(tc.tile_pool(name="meta", bufs=1))

        idxp = ctx.enter_context(tc.tile_pool(name="idxp", bufs=4))
        # max subtiles in one gather call (for gt buffer shape)
        stmax = 0
        for li in range(2):
            for cs in layouts[li][1]:
                stmax = max(stmax, cs // 128)
        gpool = ctx.enter_context(tc.tile_pool(name="gpool", bufs=3))
        xsp = ctx.enter_context(tc.tile_pool(name="xsp", bufs=2 * SG))
        ppool = ctx.enter_context(tc.tile_pool(name="ppool", bufs=6))
        upool = ctx.enter_context(tc.tile_pool(name="upool", bufs=3))
        outp = ctx.enter_context(tc.tile_pool(name="outp", bufs=3))
        psa = ctx.enter_context(tc.tile_pool(name="psa", bufs=1, space="PSUM"))
        pso = ctx.enter_context(tc.tile_pool(name="pso", bufs=1, space="PSUM"))

        def layer(li, to_r2):
            st, (sub_off, call_slots, sttot_l) = ST[li], layouts[li]
            mtot = sttot_l + g.nt
            msplit = min(int(sub_off[min(SG, g.nt - 1), 0]), mtot)
            dlt = meta.tile([128, mtot], dt.float32, tag="dl")
            wv = meta.tile([128, mtot], dt.float32, tag="wv")
            for a, b in ((0, msplit), (msplit, mtot)):
                if a >= b:
                    continue
                nc.sync.dma_start(dlt[:, a:b], dl_d[li][:, a:b])
                nc.sync.dma_start(wv[:, a:b], degp_d[li][:, a:b])
                nc.vector.reciprocal(wv[:, a:b], wv[:, a:b])
                nc.scalar.sqrt(wv[:, a:b], wv[:, a:b])
            self_src = xself_d if li == 0 else r2shard[:]

            def src_sec(s):
                if li == 0:
                    return x2[s * g.sec : (s + 1) * g.sec, :]
                if s < NSEC - 1:
                    return r2cat0[s * g.sec : (s + 1) * g.sec, :]
                return r2cat1[:]

            for gsg in range(g.nsg):
                t0, t1 = gsg * SG, min((gsg + 1) * SG, g.nt)
                accs = {}
                xs = {}
                for t in range(t0, t1):
                    accs[t] = psa.tile(
                        [128, 512], dt.float32, tag=f"acc{t - t0}",
                        name=f"acc{t - t0}",
                    )
                    xs[t] = xsp.tile(
                        [128, DF], dt.bfloat16, tag="xs", name="xs"
                    )
                    e = min(128, g.ldim - t * TW)
                    nc.sync.dma_start(
                        xs[t][0:e, :], self_src[t * TW : t * TW + e, :]
                    )
                for s in range(NSEC):
                    slots = call_slots[gsg * NSEC + s]
                    if slots == 0:
                        continue
                    nsub = slots // 128
                    base_sub = sub_off[t0, s]
                    it = idxp.tile([128, stmax * 8], dt.int16, tag="it")
                    nc.sync.dma_start(
                        it[:, : nsub * 8],
                        idx_d[li][:, base_sub * 8 : (base_sub + nsub) * 8],
                    )
                    gt = gpool.tile([128, stmax, DF], dt.bfloat16, tag="gt")
                    nc.gpsimd.dma_gather(
                        gt[:, :nsub, :],
                        src_sec(s),
                        it[:, : nsub * 8],
                        slots,
                        slots,
                        DF,
                        single_packet=False,
                    )
                    for t in range(t0, t1):
                        for j in range(st[t, s]):
                            col = sub_off[t, s] + j
                            off = col - base_sub
                            P = ppool.tile([128, TW], dt.bfloat16, tag="P")
                            nc.vector.tensor_scalar(
                                P[:],
                                iota_sb[:, 0:TW],
                                dlt[:, col : col + 1],
                                wv[:, col : col + 1],
                                mybir.AluOpType.is_equal,
                                mybir.AluOpType.mult,
                            )
                            first = True
                            for s2 in range(NSEC):
                                if st[t, s2] > 0:
                                    first = s2 == s and j == 0
                                    break
                            nc.tensor.matmul(
                                accs[t][:, 0:TW],
                                lhsT=gt[:, off, :],
                                rhs=P[:],
                                start=first,
                                stop=False,
                            )
                # self-loop term + post per tile
                for t in range(t0, t1):
                    col = sttot_l + t
                    e = min(128, g.ldim - t * TW)
                    P = ppool.tile([128, TW], dt.bfloat16, tag="P")
                    nc.vector.tensor_scalar(
                        P[:],
                        iota_sb[:, 0:TW],
                        dlt[:, col : col + 1],
                        wv[:, col : col + 1],
                        mybir.AluOpType.is_equal,
                        mybir.AluOpType.mult,
                    )
                    nc.tensor.matmul(
                        accs[t][:, 0:TW],
                        lhsT=xs[t][0:e, :],
                        rhs=P[0:e, :],
                        start=all(st[t, s] == 0 for s in range(NSEC)),
                        stop=True,
                    )
                    u = upool.tile([128, TW], dt.bfloat16, tag="u")
                    nc.scalar.activation(
                        u[:],
                        accs[t][:, 0:TW],
                        mybir.ActivationFunctionType.Copy,
                    )
                    po = pso.tile([128, 512], dt.float32, tag="po")
                    nc.tensor.matmul(
                        po[:, 0:TW],
                        lhsT=wc_sb[li][:],
                        rhs=u[:],
                        start=True,
                        stop=True,
                    )
                    sl = slice(t * TW, (t + 1) * TW)
                    if to_r2:
                        rb = outp.tile([128, 128], dt.bfloat16, tag="rb")
                        nc.scalar.activation(
                            rb[:, 0:TW],
                            po[:, 0:TW],
                            mybir.ActivationFunctionType.Relu,
                            bias=bc_sb[li][:],
                        )
                        rt = outp.tile([128, 128], dt.bfloat16, tag="rt")
                        nc.sync.dma_start_transpose(rt[:], rb[:])
                        nc.sync.dma_start(r2shard[sl, :], rt[0:TW, :])
                    else:
                        nc.scalar.activation(
                            z_sb[:, sl],
                            po[:, 0:TW],
                            mybir.ActivationFunctionType.Identity,
                            bias=bc_sb[li][:],
                        )
                        if t == g.nt - 1:
                            nc.vector.tensor_tensor(
                                z_sb[:, sl],
                                z_sb[:, sl],
                                lastmask_sb[:],
                                op=mybir.AluOpType.mult,
                            )

        layer(0, to_r2=True)

        nc.gpsimd.collective_compute(
            "AllGather",
            mybir.AluOpType.bypass,
            replica_groups=rg,
            ins=[r2shard[0 : g.ch0, :].opt()],
            outs=[r2cat0[:].opt()],
        )
        nc.gpsimd.collective_compute(
            "AllGather",
            mybir.AluOpType.bypass,
            replica_groups=rg,
            ins=[r2shard[g.ch0 : g.ldim, :].opt()],
            outs=[r2cat1[:].opt()],
        )

        layer(1, to_r2=False)

        # ---- DGI readout ----
        fin = ctx.enter_context(tc.tile_pool(name="fin", bufs=1))
        zsplit = (g.nsg - 2) * SG * TW
        cs_a = fin.tile([128, 1], dt.float32, tag="cs_a")
        nc.vector.reduce_sum(cs_a[:], z_sb[:, :zsplit], axis=mybir.AxisListType.X)
        cs_b = fin.tile([128, 1], dt.float32, tag="cs_b")
        nc.vector.reduce_sum(cs_b[:], z_sb[:, zsplit:], axis=mybir.AxisListType.X)
        cs = fin.tile([128, 1], dt.float32, tag="cs")
        nc.vector.tensor_tensor(cs[:], cs_a[:], cs_b[:], op=mybir.AluOpType.add)
        nc.sync.dma_start(cs_in[:], cs[:])
        nc.gpsimd.collective_compute(
            "AllReduce",
            mybir.AluOpType.add,
            replica_groups=rg,
            ins=[cs_in[:].opt()],
            outs=[cs_out[:].opt()],
        )
        cst = fin.tile([128, 1], dt.float32, tag="cst")
        nc.sync.dma_start(cst[:], cs_out[:])
        summ = fin.tile([128, 1], dt.float32, tag="summ")
        nc.scalar.activation(
            summ[:], cst[:], mybir.ActivationFunctionType.Sigmoid, scale=inv_n
        )
        wsps = pso.tile([128, 512], dt.float32, tag="po")
        nc.tensor.matmul(
            wsps[0:DF, 0:1], lhsT=wstack_sb[:], rhs=summ[0:D, 0:1],
            start=True, stop=True,
        )
        ws2 = fin.tile([DF, 2], dt.float32, tag="ws2")
        nc.vector.tensor_tensor(
            ws2[:],
            colmask_sb[:],
            wsps[0:DF, 0:1].to_broadcast([DF, 2]),
            op=mybir.AluOpType.mult,
        )
        tp_sb = fin.tile([128, g.nt], dt.float32, tag="tp_sb")
        tn_sb = fin.tile([128, g.nt], dt.float32, tag="tn_sb")
        nc.vector.memset(tp_sb[:], 0.0)
        nc.vector.memset(tn_sb[:], 0.0)
        PER_BANK = 32
        for b0 in range(0, g.nt, PER_BANK):
            nb = min(PER_BANK, g.nt - b0)
            tps = pso.tile([128, 512], dt.float32, tag="po")
            for i in range(nb):
                dti = b0 + i
                sl = slice(dti * TW, (dti + 1) * TW)
                nc.tensor.matmul(
                    tps[0:TW, 2 * i : 2 * i + 2],
                    lhsT=z_sb[:, sl],
                    rhs=ws2[:],
                    start=True,
                    stop=True,
                )
            tpv = tps[0:TW, 0 : 2 * nb].rearrange("p (a f) -> p a f", f=2)
            nc.vector.tensor_copy(
                tp_sb[0:TW, b0 : b0 + nb], tpv[:, :, 0:1].opt()
            )
            nc.vector.tensor_copy(
                tn_sb[0:TW, b0 : b0 + nb], tpv[:, :, 1:2].opt()
            )

        # softplus(sgn*t) = relu(sgn*t) + ln1p(exp(-|t|)); deg-7 poly for ln1p
        LN1P = [
            5.62195900721818e-07, 0.9999574870750696, -0.4992065685478763,
            0.32697310001391783, -0.2228362583278401, 0.13076503250360005,
            -0.05262485136716543, 0.010119082927575069,
        ]

        def softplus_of(t_in, sgn, tagp):
            neg = fin.tile([128, g.nt], dt.float32, tag=f"{tagp}neg")
            nc.vector.tensor_scalar(
                neg[:], t_in[:], -1.0, None, mybir.AluOpType.mult
            )
            ab = fin.tile([128, g.nt], dt.float32, tag=f"{tagp}ab")
            nc.vector.tensor_tensor(ab[:], t_in[:], neg[:], op=mybir.AluOpType.max)
            uu = fin.tile([128, g.nt], dt.float32, tag=f"{tagp}uu")
            nc.scalar.activation(
                uu[:], ab[:], mybir.ActivationFunctionType.Exp, scale=-1.0
            )
            pp_ = fin.tile([128, g.nt], dt.float32, tag=f"{tagp}pp")
            nc.vector.tensor_scalar(
                pp_[:], uu[:], LN1P[7], LN1P[6],
                mybir.AluOpType.mult, mybir.AluOpType.add,
            )
            pm = fin.tile([128, g.nt], dt.float32, tag=f"{tagp}pm")
            for ci in range(5, -1, -1):
                nc.vector.tensor_tensor(
                    pm[:], pp_[:], uu[:], op=mybir.AluOpType.mult
                )
                nc.vector.tensor_scalar(
                    pp_[:], pm[:], LN1P[ci], None, mybir.AluOpType.add
                )
            rl = fin.tile([128, g.nt], dt.float32, tag=f"{tagp}rl")
            nc.vector.tensor_scalar(
                rl[:], (t_in if sgn > 0 else neg)[:], 0.0, None,
                mybir.AluOpType.max,
            )
            res = fin.tile([128, g.nt], dt.float32, tag=f"{tagp}res")
            nc.vector.tensor_tensor(res[:], rl[:], pp_[:], op=mybir.AluOpType.add)
            return res

        spp = softplus_of(tp_sb, -1, "sp")   # softplus(-t_pos)
        spn = softplus_of(tn_sb, +1, "sn")   # softplus(t_neg)
        ssum = fin.tile([128, g.nt], dt.float32, tag="ssum")
        nc.vector.tensor_tensor(ssum[:], spp[:], spn[:], op=mybir.AluOpType.add)
        nc.vector.tensor_tensor(
            ssum[:], ssum[:], mask_sb[:], op=mybir.AluOpType.mult
        )
        srow = fin.tile([128, 1], dt.float32, tag="srow")
        nc.vector.reduce_sum(srow[:], ssum[:], axis=mybir.AxisListType.X)
        tot = pso.tile([128, 512], dt.float32, tag="po")
        nc.tensor.matmul(
            tot[0:1, 0:1], lhsT=srow[:], rhs=ones_sb[:], start=True, stop=True
        )
        lsb = fin.tile([1, 16], dt.float32, tag="lsb")
        nc.vector.memset(lsb[:], 0.0)
        nc.vector.tensor_copy(lsb[0:1, 0:1], tot[0:1, 0:1])
        nc.sync.dma_start(ls_in[:], lsb[:])
        nc.gpsimd.collective_compute(
            "AllReduce",
            mybir.AluOpType.add,
            replica_groups=rg,
            ins=[ls_in[:].opt()],
            outs=[ls_out[:].opt()],
        )
        lsf = fin.tile([1, 16], dt.float32, tag="lsf")
        nc.sync.dma_start(lsf[:], ls_out[:])
        lout = fin.tile([1, 16], dt.float32, tag="lout")
        nc.scalar.activation(
            lout[:], lsf[:], mybir.ActivationFunctionType.Copy, scale=inv_n
        )
        nc.sync.dma_start(loss_out, lout[:])

    nc.compile()
    return nc


_prog_cache = {}


def _get_prog(g, ST, layouts, meta):
    key = (g.npc, g.nreal, meta)
    if key not in _prog_cache:
        _prog_cache[key] = _build(g, ST, layouts)
    return _prog_cache[key]


def run(inputs, npc=12500, nreal=100000, trace=False):
    g = Geo(npc, nreal)
    in_maps, (ST, layouts, meta) = _preprocess(g, **inputs)
    nc = _get_prog(g, ST, layouts, meta)
    res = run_bass_kernel_spmd(
        nc, in_maps, core_ids=list(range(C)), trace=trace
    )
    loss = res.results[0]["loss"][0, 0]
    return np.float32(loss), res


def kernel(**inputs):
    out, _ = run(inputs)
    return out


def _make_sharded_exec(nc, in_maps, reps=1):
    """Jitted shard_map executor mirroring bass2jax's multi-core path, with
    device-resident inputs."""
    import jax
    from jax.experimental.shard_map import shard_map
    from jax.sharding import Mesh, NamedSharding, PartitionSpec

    from concourse import bass2jax, mybir as _mb

    bass2jax.install_neuronx_cc_hook()
    partition_name = (
        nc.partition_id_tensor.name if nc.partition_id_tensor else None
    )
    in_names, out_names, out_avals, zero_shapes = [], [], [], []
    for alloc in nc.m.functions[0].allocations:
        if not isinstance(alloc, _mb.MemoryLocationSet):
            continue
        name = alloc.memorylocations[0].name
        if alloc.kind == "ExternalInput":
            if name != partition_name:
                in_names.append(name)
        elif alloc.kind == "ExternalOutput":
            shape = tuple(alloc.tensor_shape)
            dty = _mb.dt.np(alloc.dtype)
            out_names.append(name)
            out_avals.append(jax.core.ShapedArray(shape, dty))
            zero_shapes.append((shape, dty))
    n_params = len(in_names)
    n_outs = len(out_avals)
    all_names = list(in_names) + list(out_names)
    if partition_name is not None:
        all_names.append(partition_name)
    donate = tuple(range(n_params, n_params + n_outs * reps))

    assert reps == 1

    def _body(*args):
        operands = list(args)
        if partition_name is not None:
            operands.append(bass2jax.partition_id_tensor())
        outs = bass2jax._bass_exec_p.bind(
            *operands,
            out_avals=tuple(out_avals),
            in_names=tuple(all_names),
            out_names=tuple(out_names),
            lowering_input_output_aliases=(),
            sim_require_finite=True,
            sim_require_nnan=True,
            nc=nc,
        )
        return tuple(outs)

    devices = jax.devices()[:C]
    mesh = Mesh(np.array(devices), ("core",))
    spec = PartitionSpec("core")
    sharded = jax.jit(
        shard_map(
            _body,
            mesh=mesh,
            in_specs=(spec,) * (n_params + n_outs * reps),
            out_specs=(spec,) * n_outs,
            check_rep=False,
        ),
        donate_argnums=donate,
        keep_unused=True,
    )
    shard = NamedSharding(mesh, spec)
    concat_in = [
        jax.device_put(
            np.concatenate([np.asarray(m[nm]) for m in in_maps], axis=0), shard
        )
        for nm in in_names
    ]

    def launch():
        zeros = [
            jax.device_put(np.zeros((C * s[0], *s[1:]), d), shard)
            for (s, d) in zero_shapes
        ]
        return sharded(*concat_in, *zeros)

    def fetch(outs):
        jax.block_until_ready(outs)
        return {
            nm: np.asarray(outs[i]).reshape(C, *out_avals[i].shape)[0]
            for i, nm in enumerate(out_names)
        }

    def run_once():
        return fetch(launch())

    run_once.launch = launch
    run_once.fetch = fetch
    return run_once


def bench(inputs, npc=12500, nreal=100000, iters=6):
    import time

    g = Geo(npc, nreal)
    t0 = time.time()
    in_maps, (ST, layouts, meta) = _preprocess(g, **inputs)
    t1 = time.time()
    nc = _get_prog(g, ST, layouts, meta)
    t2 = time.time()
    run_1 = _make_sharded_exec(nc, in_maps)
    out = run_1()  # warmup: compiles + loads NEFF
    t3 = time.time()
    t1s = []
    for _ in range(iters):
        ta = time.time()
        out = run_1()
        t1s.append(time.time() - ta)
    # Pipelined-launch marginal: executions overlap the ~150ms axon
    # round-trip, so the added time per extra launch approximates on-device
    # execution. The kernel is now near the measurement noise floor
    # (a trivial NEFF measures ~0.4ms by the same method), so take the
    # median of several repetitions.
    import statistics
    import jax as _jax
    pers = []
    for _rep in range(9):
        K = 16
        ta = time.time()
        pend = [run_1.launch() for _ in range(K)]
        _jax.block_until_ready(pend)
        tK = time.time() - ta
        pers.append((tK - min(t1s)) / (K - 1))
    per = max(statistics.median(pers), 1e-6)
    print(f"  marginal reps ms: {[round(p*1e3,3) for p in pers]}")
    print(
        f"preprocess {t1-t0:.1f}s  build {t2-t1:.1f}s  warmup {t3-t2:.1f}s\n"
        f"  1-shot ms: {[round(t*1e3,2) for t in t1s]}\n"
        f"  {K} pipelined: total {tK*1e3:.1f} ms -> marginal {per*1e3:.3f} ms"
    )
    return np.float32(out["loss"][0, 0]), per
